# revision 11
# baseline (speedup 1.0000x reference)
"""Trainium2 Bass kernel for nn_AE_30142080483951 (gnn_message_passing).

Data-parallel over batch B=8 across 8 NeuronCores (one batch element per
core, weights replicated, no collectives).  Key restructuring vs the
reference:

  - The (M,M) affinity matrix A = SPf^T @ sigma @ SPf is rank-64, so
    A @ yT is computed as SPf^T @ (sigma @ (SPf @ yT)) without ever
    materializing A; the gnn linear is folded into the same low-rank chain.
  - softmax(sp_adj) @ yT is computed from the host-transposed adjacency
    ST = sp_adj.T streamed tile-by-tile: exp on ScalarE, row-normalizer
    via a ones-matmul column sum, the division deferred to the (Ci, M)
    output.
  - BatchNorms are folded to per-channel scale/bias applied by ScalarE
    activations straight out of PSUM.
  - bf16 compute on the TensorEngine (rel tolerance 2e-2), fp32 PSUM
    accumulation and fp32 residual/activation chain.
"""

import numpy as np
from contextlib import ExitStack

EPS = 1e-5
B, N, Cs, Cin, Ci, Co = 8, 48, 64, 256, 128, 128
M = N * N            # 2304
MT = M // 128        # 18 token tiles
HW = (2 * N) * (2 * N)  # 9216
HWH = HW // 2        # 4608 (one image row-half per partition group)
CH = [(0, 512), (512, 512), (1024, 512), (1536, 512), (2048, 256)]

_CACHE = {}


def _split_sync_waits(nc):
    """This toolchain's walrus rejects instructions carrying more than 2 sem
    waits (1 for TPB_CTRL ops like Drain/NoOp).  Move excess waits onto
    same-engine NoOps inserted just before the offending instruction."""
    import concourse.mybir as mybir

    ctr = 0
    for f in nc.m.functions:
        for blk in f.blocks:
            changed = False
            out = []
            for inst in blk.instructions:
                si = getattr(inst, "sync_info", None)
                waits = list(si.on_wait) if si is not None else []
                op = str(getattr(inst, "opcode", ""))
                # DMA/collective ops run on their own queue processors — a
                # same-engine NoOp does NOT gate them, so leave their waits
                # alone (the descriptor encoding allows more waits anyway).
                if "DMA" in op or "Collective" in op or "Dma" in op:
                    out.append(inst)
                    continue
                lim = 1
                if len(waits) > lim:
                    si.on_wait = waits[:lim]
                    for w in waits[lim:]:
                        ctr += 1
                        nop = mybir.InstNoOp(name=f"I-wsplit{ctr}", ins=[], outs=[])
                        nop.engine = inst.engine
                        nop.sync_info = mybir.SyncInfo(on_wait=[w], on_update=[])
                        nc.register_instruction(nop, overwrite=True)
                        out.append(nop)
                    changed = True
                out.append(inst)
            if changed:
                blk.instructions = out


def _build():
    import concourse.bass as bass
    import concourse.bacc as bacc_mod
    import concourse.mybir as mybir
    import concourse.tile as tile
    from concourse.bass import MemorySpace
    from concourse.masks import make_identity

    f32 = mybir.dt.float32
    bf = mybir.dt.bfloat16
    AF = mybir.ActivationFunctionType

    nc = bacc_mod.Bacc("TRN2")

    # ---- DRAM parameters (per-core shard; bf16 for matmul operands) ----
    x_d = nc.dram_tensor("x", [Cin, M], bf, kind="ExternalInput")
    sp_d = nc.dram_tensor("sp", [Cs, HW], bf, kind="ExternalInput")
    st_d = nc.dram_tensor("st", [M, M], bf, kind="ExternalInput")
    w1t_d = nc.dram_tensor("w1t", [Cin, Ci], bf, kind="ExternalInput")
    wnct_d = nc.dram_tensor("wnct", [M, Cs], bf, kind="ExternalInput")
    bnc_d = nc.dram_tensor("bnc", [1, Cs], bf, kind="ExternalInput")
    wkct_d = nc.dram_tensor("wkct", [Ci, Cs], bf, kind="ExternalInput")
    gnnwt_d = nc.dram_tensor("gnnwt", [Ci, Ci], bf, kind="ExternalInput")
    spwt_d = nc.dram_tensor("spwt", [Ci, Ci], bf, kind="ExternalInput")
    backwt_d = nc.dram_tensor("backwt", [Ci, Co], bf, kind="ExternalInput")
    bn1s_d = nc.dram_tensor("bn1s", [Ci, 1], f32, kind="ExternalInput")
    bn1b_d = nc.dram_tensor("bn1b", [Ci, 1], f32, kind="ExternalInput")
    bkc_d = nc.dram_tensor("bkc", [Cs, 1], f32, kind="ExternalInput")
    gnnb_d = nc.dram_tensor("gnnb", [Ci, 1], f32, kind="ExternalInput")
    spb_d = nc.dram_tensor("spb", [Ci, 1], f32, kind="ExternalInput")
    bn2s_d = nc.dram_tensor("bn2s", [Co, 1], f32, kind="ExternalInput")
    bn2b_d = nc.dram_tensor("bn2b", [Co, 1], f32, kind="ExternalInput")
    out_d = nc.dram_tensor("out", [Co, M], f32, kind="ExternalOutput")

    tc = tile.TileContext(nc)
    with tc:
        with ExitStack() as ctx:
            ctx.enter_context(
                nc.allow_low_precision(reason="bf16 compute path, rel tol 2e-2")
            )
            singles = ctx.enter_context(tc.tile_pool(name="singles", bufs=1))
            stream = ctx.enter_context(tc.tile_pool(name="stream", bufs=4))
            chunks = ctx.enter_context(tc.tile_pool(name="chunks", bufs=3))
            psA = ctx.enter_context(
                tc.tile_pool(name="psA", bufs=4, space=MemorySpace.PSUM)
            )
            psT = ctx.enter_context(
                tc.tile_pool(name="psT", bufs=2, space=MemorySpace.PSUM)
            )
            psS = ctx.enter_context(
                tc.tile_pool(name="psS", bufs=2, space=MemorySpace.PSUM)
            )

            # ---- constants / weights to SBUF ----
            w1t_sb = singles.tile([128, 2, Ci], bf)
            nc.sync.dma_start(
                out=w1t_sb[:], in_=w1t_d[:, :].rearrange("(kt p) c -> p kt c", p=128)
            )
            x_sb = singles.tile([128, 2, M], bf)
            nc.sync.dma_start(
                out=x_sb[:], in_=x_d[:, :].rearrange("(kt p) m -> p kt m", p=128)
            )
            wnct_sb = singles.tile([128, MT, Cs], bf)
            nc.sync.dma_start(
                out=wnct_sb[:], in_=wnct_d[:, :].rearrange("(ti p) c -> p ti c", p=128)
            )
            wkct_sb = singles.tile([Ci, Cs], bf)
            nc.sync.dma_start(out=wkct_sb[:], in_=wkct_d[:, :])
            gnnwt_sb = singles.tile([Ci, Ci], bf)
            nc.sync.dma_start(out=gnnwt_sb[:], in_=gnnwt_d[:, :])
            spwt_sb = singles.tile([Ci, Ci], bf)
            nc.sync.dma_start(out=spwt_sb[:], in_=spwt_d[:, :])
            backwt_sb = singles.tile([Ci, Co], bf)
            nc.sync.dma_start(out=backwt_sb[:], in_=backwt_d[:, :])
            bnc_sb = singles.tile([1, Cs], bf)
            nc.sync.dma_start(out=bnc_sb[:], in_=bnc_d[:, :])

            bn1s_sb = singles.tile([Ci, 1], f32)
            nc.sync.dma_start(out=bn1s_sb[:], in_=bn1s_d[:, :])
            bn1b_sb = singles.tile([Ci, 1], f32)
            nc.sync.dma_start(out=bn1b_sb[:], in_=bn1b_d[:, :])
            bkc_sb = singles.tile([Cs, 1], f32)
            nc.sync.dma_start(out=bkc_sb[:], in_=bkc_d[:, :])
            gnnb_sb = singles.tile([Ci, 1], f32)
            nc.sync.dma_start(out=gnnb_sb[:], in_=gnnb_d[:, :])
            spb_sb = singles.tile([Ci, 1], f32)
            nc.sync.dma_start(out=spb_sb[:], in_=spb_d[:, :])
            bn2s_sb = singles.tile([Co, 1], f32)
            nc.sync.dma_start(out=bn2s_sb[:], in_=bn2s_d[:, :])
            bn2b_sb = singles.tile([Co, 1], f32)
            nc.sync.dma_start(out=bn2b_sb[:], in_=bn2b_d[:, :])

            ident = singles.tile([128, 128], bf)
            make_identity(nc, ident[:])
            ones1 = singles.tile([1, 128], bf)
            nc.vector.memset(ones1[:], 1.0)
            onesP = singles.tile([128, 128], bf)
            nc.vector.memset(onesP[:], 1.0)
            sel = singles.tile([128, 2], bf)
            nc.vector.memset(sel[:], 0.0)
            nc.vector.memset(sel[0:64, 0:1], 1.0)
            nc.vector.memset(sel[64:128, 1:2], 1.0)

            # ---- SP branch: channel softmax + 2x2 maxpool ----
            # sp_sb partition p = h*64 + c  (h = image row-half), free = 48x96 px
            sp_sb = singles.tile([128, HWH], bf)
            nc.sync.dma_start(out=sp_sb[0:64, :], in_=sp_d[:, 0:HWH])
            nc.sync.dma_start(out=sp_sb[64:128, :], in_=sp_d[:, HWH:HW])
            # E = exp(sp) in place
            nc.scalar.activation(sp_sb[:], sp_sb[:], AF.Exp)
            # per-pixel channel sums via sel-matmul; reciprocal into d_sb
            d_sb = singles.tile([2, HWH], bf)
            for jo in range(0, HWH, 512):
                ps = psS.tile([2, 512], f32, tag="ps_small")
                nc.tensor.matmul(ps[:], sel[:], sp_sb[:, jo : jo + 512])
                nc.vector.reciprocal(d_sb[:, jo : jo + 512], ps[:])
            # broadcast inverse-sums to all (h, c) partitions
            d_dram = nc.dram_tensor("d_scratch", [2, HWH], bf, kind="Internal")
            nc.sync.dma_start(out=d_dram[:, :], in_=d_sb[:])
            d_rep = singles.tile([128, HWH], bf)
            nc.gpsimd.dma_start(
                out=d_rep[0:64, :], in_=d_dram[0:1, :].to_broadcast((64, HWH))
            )
            nc.gpsimd.dma_start(
                out=d_rep[64:128, :], in_=d_dram[1:2, :].to_broadcast((64, HWH))
            )
            nc.vector.tensor_mul(sp_sb[:], sp_sb[:], d_rep[:])
            # maxpool 2x2: f = rp*192 + dy*96 + qp*2 + dx
            pooled = singles.tile([128, 1152], bf)
            nc.vector.tensor_reduce(
                out=pooled[:].rearrange("p (rp qp) -> p rp qp", rp=24),
                in_=sp_sb[:].rearrange(
                    "p (rp dy qp dx) -> p rp qp dy dx", rp=24, dy=2, qp=48, dx=2
                ),
                axis=mybir.AxisListType.XY,
                op=mybir.AluOpType.max,
            )
            # reassemble channel-major SPf (64, 2304)
            spf_sb = singles.tile([Cs, M], bf)
            nc.sync.dma_start(out=spf_sb[:, 0:1152], in_=pooled[0:64, :])
            nc.sync.dma_start(out=spf_sb[:, 1152:2304], in_=pooled[64:128, :])

            # ---- t = relu(bn1(W1 @ x)), channel-major (128, 2304) ----
            t_sb = singles.tile([Ci, M], bf)
            for mo, mw in CH:
                ps = psA.tile([128, 512], f32, tag="ps_big")
                nc.tensor.matmul(
                    ps[:, :mw],
                    w1t_sb[:, 0, :],
                    x_sb[:, 0, mo : mo + mw],
                    start=True,
                    stop=False,
                )
                nc.tensor.matmul(
                    ps[:, :mw],
                    w1t_sb[:, 1, :],
                    x_sb[:, 1, mo : mo + mw],
                    start=False,
                    stop=True,
                )
                nc.scalar.activation(
                    t_sb[:, mo : mo + mw],
                    ps[:, :mw],
                    AF.Relu,
                    bias=bn1b_sb[:],
                    scale=bn1s_sb[:],
                )

            # ---- yT (token-major) via PE transposes ----
            yT_sb = singles.tile([128, MT, Ci], bf)
            for ti in range(MT):
                ps = psT.tile([128, 128], bf, tag="ps_tr")
                nc.tensor.transpose(
                    ps[:], t_sb[:, ti * 128 : (ti + 1) * 128], ident[:]
                )
                nc.vector.tensor_copy(yT_sb[:, ti, :], ps[:])

            # ---- SPfT (token-major) via PE transposes ----
            spfT_sb = singles.tile([128, MT, Cs], bf)
            for ti in range(MT):
                ps = psT.tile([128, 64], bf, tag="ps_tr")
                nc.tensor.transpose(
                    ps[:], spf_sb[:, ti * 128 : (ti + 1) * 128], ident[0:64, 0:64]
                )
                nc.vector.tensor_copy(spfT_sb[:, ti, :], ps[:])

            # ---- yc (k, c) = t @ WncT + bnc ----
            ps_yc = psS.tile([128, Cs], f32, tag="ps_small")
            for ti in range(MT):
                nc.tensor.matmul(
                    ps_yc[:],
                    yT_sb[:, ti, :],
                    wnct_sb[:, ti, :],
                    start=(ti == 0),
                    stop=False,
                )
            nc.tensor.matmul(ps_yc[:], ones1[:], bnc_sb[:], start=False, stop=True)
            yc_sb = singles.tile([Ci, Cs], bf)
            nc.vector.tensor_copy(yc_sb[:], ps_yc[:])

            # ---- sigT (d, c) = Wkc @ yc + bkc ----
            ps_sg = psS.tile([Cs, Cs], f32, tag="ps_small")
            nc.tensor.matmul(ps_sg[:], wkct_sb[:], yc_sb[:])
            sigT_sb = singles.tile([Cs, Cs], bf)
            nc.scalar.activation(sigT_sb[:], ps_sg[:], AF.Identity, bias=bkc_sb[:])

            # ---- G (d, k) = SPf @ yT ----
            ps_g = psS.tile([Cs, Ci], f32, tag="ps_small")
            for ti in range(MT):
                nc.tensor.matmul(
                    ps_g[:],
                    spfT_sb[:, ti, :],
                    yT_sb[:, ti, :],
                    start=(ti == 0),
                    stop=(ti == MT - 1),
                )
            g_sb = singles.tile([Cs, Ci], bf)
            nc.vector.tensor_copy(g_sb[:], ps_g[:])

            # ---- HT (k, c) = G.T @ sigT ----
            ps_ht = psS.tile([Ci, Cs], f32, tag="ps_small")
            nc.tensor.matmul(ps_ht[:], g_sb[:], sigT_sb[:])
            ht_sb = singles.tile([Ci, Cs], bf)
            nc.vector.tensor_copy(ht_sb[:], ps_ht[:])

            # ---- HG (c, k2) = HT.T @ gnn_wT ----
            ps_hg = psS.tile([Cs, Ci], f32, tag="ps_small")
            nc.tensor.matmul(ps_hg[:], ht_sb[:], gnnwt_sb[:])
            hg_sb = singles.tile([Cs, Ci], bf)
            nc.vector.tensor_copy(hg_sb[:], ps_hg[:])

            # ---- main chunk loop: se linear, sp branch, combine, back ----
            for mo, mw in CH:
                # relu_se chunk = relu(HG.T @ SPf + gnn_b)
                ps_se = psA.tile([128, 512], f32, tag="ps_big")
                nc.tensor.matmul(ps_se[:, :mw], hg_sb[:], spf_sb[:, mo : mo + mw])
                rse = chunks.tile([128, 512], f32, tag="rse")
                nc.scalar.activation(
                    rse[:, :mw], ps_se[:, :mw], AF.Relu, bias=gnnb_sb[:]
                )

                # sp_preT chunk: stream 18 n-tiles of exp(ST)
                ps_sp = psA.tile([128, 512], f32, tag="ps_big")
                ps_cs = psA.tile([128, 512], f32, tag="ps_big")
                for i in range(MT):
                    st_t = stream.tile([128, 512], bf, tag="st")
                    nc.sync.dma_start(
                        out=st_t[:, :mw],
                        in_=st_d[i * 128 : (i + 1) * 128, mo : mo + mw],
                    )
                    est = stream.tile([128, 512], bf, tag="est")
                    nc.scalar.activation(est[:, :mw], st_t[:, :mw], AF.Exp)
                    nc.tensor.matmul(
                        ps_sp[:, :mw],
                        yT_sb[:, i, :],
                        est[:, :mw],
                        start=(i == 0),
                        stop=(i == MT - 1),
                    )
                    nc.tensor.matmul(
                        ps_cs[:, :mw],
                        onesP[:],
                        est[:, :mw],
                        start=(i == 0),
                        stop=(i == MT - 1),
                    )
                rrep = chunks.tile([128, 512], f32, tag="rrep")
                nc.vector.reciprocal(rrep[:, :mw], ps_cs[:, :mw])
                spre = chunks.tile([128, 512], bf, tag="spre")
                nc.vector.tensor_mul(spre[:, :mw], ps_sp[:, :mw], rrep[:, :mw])

                # relu_sp chunk = relu(sp_w @ sp_preT + sp_b)
                ps_sl = psA.tile([128, 512], f32, tag="ps_big")
                nc.tensor.matmul(ps_sl[:, :mw], spwt_sb[:], spre[:, :mw])
                y3a = chunks.tile([128, 512], f32, tag="y3a")
                nc.scalar.activation(
                    y3a[:, :mw], ps_sl[:, :mw], AF.Relu, bias=spb_sb[:]
                )

                # y3 = relu_se + relu_sp + 3t
                nc.vector.tensor_add(y3a[:, :mw], y3a[:, :mw], rse[:, :mw])
                y3b = chunks.tile([128, 512], bf, tag="y3b")
                nc.vector.scalar_tensor_tensor(
                    out=y3b[:, :mw],
                    in0=t_sb[:, mo : mo + mw],
                    scalar=3.0,
                    in1=y3a[:, :mw],
                    op0=mybir.AluOpType.mult,
                    op1=mybir.AluOpType.add,
                )

                # back: relu(bn2(back_w @ y3))
                ps_bk = psA.tile([128, 512], f32, tag="ps_big")
                nc.tensor.matmul(ps_bk[:, :mw], backwt_sb[:], y3b[:, :mw])
                ob = chunks.tile([128, 512], f32, tag="ob")
                nc.scalar.activation(
                    ob[:, :mw],
                    ps_bk[:, :mw],
                    AF.Relu,
                    bias=bn2b_sb[:],
                    scale=bn2s_sb[:],
                )
                nc.sync.dma_start(out=out_d[:, mo : mo + mw], in_=ob[:, :mw])

    nc.finalize()
    return nc


def _host_prep(inputs):
    """Fold BNs, transpose weights, cast matmul operands to bf16, build
    the 8 per-core input maps (core b gets batch element b)."""
    import ml_dtypes

    f = np.float32
    bf = ml_dtypes.bfloat16
    x = np.ascontiguousarray(inputs["x"], dtype=f).reshape(B, Cin, M)
    SP = np.ascontiguousarray(inputs["SP"], dtype=f).reshape(B, Cs, HW)

    bn1s = (np.asarray(inputs["bn1_gamma"]) / np.sqrt(np.asarray(inputs["bn1_var"]) + EPS)).astype(f)
    bn1b = (np.asarray(inputs["bn1_beta"]) - np.asarray(inputs["bn1_mean"]) * bn1s).astype(f)
    bn2s = (np.asarray(inputs["bn2_gamma"]) / np.sqrt(np.asarray(inputs["bn2_var"]) + EPS)).astype(f)
    bn2b = (np.asarray(inputs["bn2_beta"]) - np.asarray(inputs["bn2_mean"]) * bn2s).astype(f)

    shared = {
        "st": np.ascontiguousarray(np.asarray(inputs["sp_adj"]).T).astype(bf),
        "w1t": np.ascontiguousarray(np.asarray(inputs["trans_w"]).T).astype(bf),
        "wnct": np.ascontiguousarray(np.asarray(inputs["linNC_w"]).T).astype(bf),
        "bnc": np.asarray(inputs["linNC_b"], dtype=f).reshape(1, Cs).astype(bf),
        "wkct": np.ascontiguousarray(np.asarray(inputs["linKC_w"]).T).astype(bf),
        "gnnwt": np.ascontiguousarray(np.asarray(inputs["gnn_w"]).T).astype(bf),
        "spwt": np.ascontiguousarray(np.asarray(inputs["sp_w"]).T).astype(bf),
        "backwt": np.ascontiguousarray(np.asarray(inputs["back_w"]).T).astype(bf),
        "bn1s": bn1s.reshape(Ci, 1),
        "bn1b": bn1b.reshape(Ci, 1),
        "bkc": np.asarray(inputs["linKC_b"], dtype=f).reshape(Cs, 1),
        "gnnb": np.asarray(inputs["gnn_b"], dtype=f).reshape(Ci, 1),
        "spb": np.asarray(inputs["sp_b"], dtype=f).reshape(Ci, 1),
        "bn2s": bn2s.reshape(Co, 1),
        "bn2b": bn2b.reshape(Co, 1),
    }
    in_maps = []
    for b in range(B):
        m = dict(shared)
        m["x"] = np.ascontiguousarray(x[b]).astype(bf)
        m["sp"] = np.ascontiguousarray(SP[b]).astype(bf)
        in_maps.append(m)
    return in_maps


def _get_nc():
    if "nc" not in _CACHE:
        _CACHE["nc"] = _build()
    return _CACHE["nc"]


def run_spmd(inputs, trace=False, trace_cores=None):
    """Build (cached), run on cores 0-7, return BassKernelResults."""
    from concourse.bass_utils import run_bass_kernel_spmd

    nc = _get_nc()
    in_maps = _host_prep(inputs)
    kwargs = {}
    if trace:
        kwargs = dict(trace=True, trace_cores=trace_cores or [0])
    return run_bass_kernel_spmd(nc, in_maps, core_ids=list(range(8)), **kwargs)


def kernel(**inputs):
    res = run_spmd(inputs)
    out = np.stack([r["out"].reshape(Co, N, N) for r in res.results])
    return out.astype(np.float32)


# revision 12
# speedup vs baseline: 1.3004x; 1.3004x over previous
"""Trainium2 Bass kernel for nn_AE_30142080483951 (gnn_message_passing).

Data-parallel over batch B=8 across 8 NeuronCores (one batch element per
core, weights replicated, no collectives).  Key restructuring vs the
reference:

  - The (M,M) affinity matrix A = SPf^T @ sigma @ SPf is rank-64, so
    A @ yT is computed as SPf^T @ (sigma @ (SPf @ yT)) without ever
    materializing A; the gnn linear is folded into the same low-rank chain.
  - softmax(sp_adj) @ yT is computed from the host-transposed adjacency
    ST = sp_adj.T streamed in (2304, 512)-column blocks: one DMA + one
    in-place ScalarE exp per block, the row-normalizer via a ones-matmul
    column sum, the division deferred to the (Ci, M) output.
  - BatchNorms are folded to per-channel scale/bias applied by ScalarE
    activations straight out of PSUM.
  - bf16 compute on the TensorEngine (rel tolerance 2e-2), fp32 PSUM
    accumulation and fp32 residual/activation chain.
"""

import numpy as np
from contextlib import ExitStack

EPS = 1e-5
B, N, Cs, Cin, Ci, Co = 8, 48, 64, 256, 128, 128
M = N * N            # 2304
MT = M // 128        # 18 token tiles
HW = (2 * N) * (2 * N)  # 9216
HWH = HW // 2        # 4608 (one image row-half per partition group)
CH = [(0, 512), (512, 512), (1024, 512), (1536, 512), (2048, 256)]

_CACHE = {}


def _build():
    import concourse.bacc as bacc_mod
    import concourse.mybir as mybir
    import concourse.tile as tile
    from concourse.bass import MemorySpace
    from concourse.masks import make_identity

    f32 = mybir.dt.float32
    bf = mybir.dt.bfloat16
    AF = mybir.ActivationFunctionType

    nc = bacc_mod.Bacc("TRN2")

    # ---- DRAM parameters (per-core shard; bf16 for matmul operands) ----
    x_d = nc.dram_tensor("x", [Cin, M], bf, kind="ExternalInput")
    sp_d = nc.dram_tensor("sp", [Cs, HW], bf, kind="ExternalInput")
    st_d = nc.dram_tensor("st", [M, M], bf, kind="ExternalInput")
    w1t_d = nc.dram_tensor("w1t", [Cin, Ci], bf, kind="ExternalInput")
    wnct_d = nc.dram_tensor("wnct", [M, Cs], bf, kind="ExternalInput")
    bnc_d = nc.dram_tensor("bnc", [1, Cs], bf, kind="ExternalInput")
    # packed (Ci, 448) = [wkct(64) | gnnwt(128) | spwt(128) | backwt(128)]
    wpack_d = nc.dram_tensor("wpack", [Ci, 448], bf, kind="ExternalInput")
    # packed (Ci, 6) = [bn1s bn1b gnnb spb bn2s bn2b]
    bias_d = nc.dram_tensor("biases", [Ci, 6], f32, kind="ExternalInput")
    bkc_d = nc.dram_tensor("bkc", [Cs, 1], f32, kind="ExternalInput")
    out_d = nc.dram_tensor("out", [Co, M], f32, kind="ExternalOutput")

    tc = tile.TileContext(nc)
    with tc:
        with ExitStack() as ctx:
            ctx.enter_context(
                nc.allow_low_precision(reason="bf16 compute path, rel tol 2e-2")
            )
            singles = ctx.enter_context(tc.tile_pool(name="singles", bufs=1))
            psA = ctx.enter_context(
                tc.tile_pool(name="psA", bufs=4, space=MemorySpace.PSUM)
            )
            psS = ctx.enter_context(
                tc.tile_pool(name="psS", bufs=2, space=MemorySpace.PSUM)
            )

            # ---- persistent constants ----
            wpack_sb = singles.tile([Ci, 448], bf)
            nc.sync.dma_start(out=wpack_sb[:], in_=wpack_d[:, :])
            wkct_sb = wpack_sb[:, 0:64]
            gnnwt_sb = wpack_sb[:, 64:192]
            spwt_sb = wpack_sb[:, 192:320]
            backwt_sb = wpack_sb[:, 320:448]
            bias_sb = singles.tile([Ci, 6], f32)
            nc.sync.dma_start(out=bias_sb[:], in_=bias_d[:, :])
            bn1s_sb = bias_sb[:, 0:1]
            bn1b_sb = bias_sb[:, 1:2]
            gnnb_sb = bias_sb[:, 2:3]
            spb_sb = bias_sb[:, 3:4]
            bn2s_sb = bias_sb[:, 4:5]
            bn2b_sb = bias_sb[:, 5:6]
            bkc_sb = singles.tile([Cs, 1], f32)
            nc.sync.dma_start(out=bkc_sb[:], in_=bkc_d[:, :])
            bnc_sb = singles.tile([1, Cs], bf)
            nc.sync.dma_start(out=bnc_sb[:], in_=bnc_d[:, :])

            onesP = singles.tile([128, 128], bf)
            nc.vector.memset(onesP[:], 1.0)
            ones1 = onesP[0:1, :]

            # persistent activations
            spf_sb = singles.tile([Cs, M], bf)
            t_sb = singles.tile([Ci, M], bf)
            yT_sb = singles.tile([128, MT, Ci], bf)
            hg_sb = singles.tile([Cs, Ci], bf)

            # ---- phase 1: inputs -> spf, t, yT, low-rank chain -> hg ----
            with tc.tile_pool(name="phase1", bufs=1) as p1:
                ident = p1.tile([128, 128], bf)
                make_identity(nc, ident[:])
                sel = p1.tile([128, 2], bf)
                nc.vector.memset(sel[:], 0.0)
                nc.vector.memset(sel[0:64, 0:1], 1.0)
                nc.vector.memset(sel[64:128, 1:2], 1.0)

                w1t_sb = p1.tile([128, 2, Ci], bf)
                nc.sync.dma_start(
                    out=w1t_sb[:],
                    in_=w1t_d[:, :].rearrange("(kt p) c -> p kt c", p=128),
                )
                x_sb = p1.tile([128, 2, M], bf)
                nc.sync.dma_start(
                    out=x_sb[:], in_=x_d[:, :].rearrange("(kt p) m -> p kt m", p=128)
                )
                wnct_sb = p1.tile([128, MT, Cs], bf)
                nc.sync.dma_start(
                    out=wnct_sb[:],
                    in_=wnct_d[:, :].rearrange("(ti p) c -> p ti c", p=128),
                )

                # SP branch: channel softmax + 2x2 maxpool
                # sp_sb partition p = h*64 + c (h = image row-half)
                sp_sb = p1.tile([128, HWH], bf)
                nc.sync.dma_start(out=sp_sb[0:64, :], in_=sp_d[:, 0:HWH])
                nc.sync.dma_start(out=sp_sb[64:128, :], in_=sp_d[:, HWH:HW])
                nc.scalar.activation(sp_sb[:], sp_sb[:], AF.Exp)
                # per-pixel channel sums via sel-matmul; fast reciprocal
                d_sbf = p1.tile([2, HWH], f32)
                for jo in range(0, HWH, 512):
                    ps = psS.tile([2, 512], f32, tag="ps_small")
                    nc.tensor.matmul(ps[:], sel[:], sp_sb[:, jo : jo + 512])
                    nc.vector.reciprocal_approx_fast(d_sbf[:, jo : jo + 512], ps[:])
                d_dram = nc.dram_tensor("d_scratch", [2, HWH], f32, kind="Internal")
                nc.sync.dma_start(out=d_dram[:, :], in_=d_sbf[:])
                d_rep = p1.tile([128, HWH], bf)
                nc.gpsimd.dma_start(
                    out=d_rep[0:64, :], in_=d_dram[0:1, :].to_broadcast((64, HWH))
                )
                nc.gpsimd.dma_start(
                    out=d_rep[64:128, :], in_=d_dram[1:2, :].to_broadcast((64, HWH))
                )
                nc.vector.tensor_mul(sp_sb[:], sp_sb[:], d_rep[:])
                # maxpool 2x2: f = rp*192 + dy*96 + qp*2 + dx
                pooled = p1.tile([128, 1152], bf)
                nc.vector.tensor_reduce(
                    out=pooled[:].rearrange("p (rp qp) -> p rp qp", rp=24),
                    in_=sp_sb[:].rearrange(
                        "p (rp dy qp dx) -> p rp qp dy dx", rp=24, dy=2, qp=48, dx=2
                    ),
                    axis=mybir.AxisListType.XY,
                    op=mybir.AluOpType.max,
                )
                # reassemble channel-major SPf (64, 2304)
                nc.sync.dma_start(out=spf_sb[:, 0:1152], in_=pooled[0:64, :])
                nc.sync.dma_start(out=spf_sb[:, 1152:2304], in_=pooled[64:128, :])

                # t = relu(bn1(W1 @ x)), channel-major (128, 2304)
                for mo, mw in CH:
                    ps = psA.tile([128, 512], f32, tag="ps_big")
                    nc.tensor.matmul(
                        ps[:, :mw],
                        w1t_sb[:, 0, :],
                        x_sb[:, 0, mo : mo + mw],
                        start=True,
                        stop=False,
                    )
                    nc.tensor.matmul(
                        ps[:, :mw],
                        w1t_sb[:, 1, :],
                        x_sb[:, 1, mo : mo + mw],
                        start=False,
                        stop=True,
                    )
                    nc.scalar.activation(
                        t_sb[:, mo : mo + mw],
                        ps[:, :mw],
                        AF.Relu,
                        bias=bn1b_sb,
                        scale=bn1s_sb,
                    )

                # yT (token-major) via PE transposes
                for ti in range(MT):
                    ps = psS.tile([128, 128], bf, tag="ps_tr")
                    nc.tensor.transpose(
                        ps[:], t_sb[:, ti * 128 : (ti + 1) * 128], ident[:]
                    )
                    nc.vector.tensor_copy(yT_sb[:, ti, :], ps[:])

                # SPfT (token-major) via PE transposes
                spfT_sb = p1.tile([128, MT, Cs], bf)
                for ti in range(MT):
                    ps = psS.tile([128, 64], bf, tag="ps_tr")
                    nc.tensor.transpose(
                        ps[:],
                        spf_sb[:, ti * 128 : (ti + 1) * 128],
                        ident[0:64, 0:64],
                    )
                    nc.vector.tensor_copy(spfT_sb[:, ti, :], ps[:])

                # yc (k, c) = t @ WncT + bnc
                ps_yc = psS.tile([128, Cs], f32, tag="ps_small")
                for ti in range(MT):
                    nc.tensor.matmul(
                        ps_yc[:],
                        yT_sb[:, ti, :],
                        wnct_sb[:, ti, :],
                        start=(ti == 0),
                        stop=False,
                    )
                nc.tensor.matmul(ps_yc[:], ones1, bnc_sb[:], start=False, stop=True)
                yc_sb = p1.tile([Ci, Cs], bf)
                nc.vector.tensor_copy(yc_sb[:], ps_yc[:])

                # sigT (d, c) = Wkc @ yc + bkc
                ps_sg = psS.tile([Cs, Cs], f32, tag="ps_small")
                nc.tensor.matmul(ps_sg[:], wkct_sb, yc_sb[:])
                sigT_sb = p1.tile([Cs, Cs], bf)
                nc.scalar.activation(sigT_sb[:], ps_sg[:], AF.Identity, bias=bkc_sb[:])

                # G (d, k) = SPf @ yT
                ps_g = psS.tile([Cs, Ci], f32, tag="ps_small")
                for ti in range(MT):
                    nc.tensor.matmul(
                        ps_g[:],
                        spfT_sb[:, ti, :],
                        yT_sb[:, ti, :],
                        start=(ti == 0),
                        stop=(ti == MT - 1),
                    )
                g_sb = p1.tile([Cs, Ci], bf)
                nc.vector.tensor_copy(g_sb[:], ps_g[:])

                # HT (k, c) = G.T @ sigT
                ps_ht = psS.tile([Ci, Cs], f32, tag="ps_small")
                nc.tensor.matmul(ps_ht[:], g_sb[:], sigT_sb[:])
                ht_sb = p1.tile([Ci, Cs], bf)
                nc.vector.tensor_copy(ht_sb[:], ps_ht[:])

                # HG (c, k2) = HT.T @ gnn_wT
                ps_hg = psS.tile([Cs, Ci], f32, tag="ps_small")
                nc.tensor.matmul(ps_hg[:], ht_sb[:], gnnwt_sb)
                nc.vector.tensor_copy(hg_sb[:], ps_hg[:])

            # ---- main chunk loop: se linear, sp branch, combine, back ----
            with (
                tc.tile_pool(name="stream", bufs=2) as stream,
                tc.tile_pool(name="chunks", bufs=2) as chunks,
            ):
                for mo, mw in CH:
                    # one DMA + one in-place exp for the whole column block
                    est = stream.tile([128, MT, 512], bf, tag="stj")
                    nc.sync.dma_start(
                        out=est[:, :, :mw],
                        in_=st_d[:, mo : mo + mw].rearrange(
                            "(i p) m -> p i m", p=128
                        ),
                    )
                    nc.scalar.activation(
                        est[:, :, :mw], est[:, :, :mw], AF.Exp
                    )

                    # relu_se chunk = relu(HG.T @ SPf + gnn_b)
                    ps_se = psA.tile([128, 512], f32, tag="ps_big")
                    nc.tensor.matmul(
                        ps_se[:, :mw], hg_sb[:], spf_sb[:, mo : mo + mw]
                    )
                    rse = chunks.tile([128, 512], f32, tag="rse")
                    nc.scalar.activation(
                        rse[:, :mw], ps_se[:, :mw], AF.Relu, bias=gnnb_sb
                    )

                    # sp_preT chunk: main matmuls, then colsum matmuls
                    ps_sp = psA.tile([128, 512], f32, tag="ps_big")
                    for i in range(MT):
                        nc.tensor.matmul(
                            ps_sp[:, :mw],
                            yT_sb[:, i, :],
                            est[:, i, :mw],
                            start=(i == 0),
                            stop=(i == MT - 1),
                        )
                    ps_cs = psA.tile([128, 512], f32, tag="ps_big")
                    for i in range(MT):
                        nc.tensor.matmul(
                            ps_cs[:, :mw],
                            onesP[:],
                            est[:, i, :mw],
                            start=(i == 0),
                            stop=(i == MT - 1),
                        )
                    rrep = chunks.tile([128, 512], f32, tag="rrep")
                    nc.vector.reciprocal_approx_fast(rrep[:, :mw], ps_cs[:, :mw])
                    spre = chunks.tile([128, 512], bf, tag="spre")
                    nc.vector.tensor_mul(spre[:, :mw], ps_sp[:, :mw], rrep[:, :mw])

                    # relu_sp chunk = relu(sp_w @ sp_preT + sp_b)
                    ps_sl = psA.tile([128, 512], f32, tag="ps_big")
                    nc.tensor.matmul(ps_sl[:, :mw], spwt_sb, spre[:, :mw])
                    y3a = chunks.tile([128, 512], f32, tag="y3a")
                    nc.scalar.activation(
                        y3a[:, :mw], ps_sl[:, :mw], AF.Relu, bias=spb_sb
                    )

                    # y3 = relu_se + relu_sp + 3t
                    nc.vector.tensor_add(y3a[:, :mw], y3a[:, :mw], rse[:, :mw])
                    y3b = chunks.tile([128, 512], bf, tag="y3b")
                    nc.vector.scalar_tensor_tensor(
                        out=y3b[:, :mw],
                        in0=t_sb[:, mo : mo + mw],
                        scalar=3.0,
                        in1=y3a[:, :mw],
                        op0=mybir.AluOpType.mult,
                        op1=mybir.AluOpType.add,
                    )

                    # back: relu(bn2(back_w @ y3))
                    ps_bk = psA.tile([128, 512], f32, tag="ps_big")
                    nc.tensor.matmul(ps_bk[:, :mw], backwt_sb, y3b[:, :mw])
                    ob = chunks.tile([128, 512], f32, tag="ob")
                    nc.scalar.activation(
                        ob[:, :mw],
                        ps_bk[:, :mw],
                        AF.Relu,
                        bias=bn2b_sb,
                        scale=bn2s_sb,
                    )
                    nc.sync.dma_start(out=out_d[:, mo : mo + mw], in_=ob[:, :mw])

    nc.finalize()
    return nc


def _host_prep(inputs):
    """Fold BNs, transpose weights, cast matmul operands to bf16, build
    the 8 per-core input maps (core b gets batch element b)."""
    import ml_dtypes

    f = np.float32
    bf = ml_dtypes.bfloat16
    x = np.ascontiguousarray(inputs["x"], dtype=f).reshape(B, Cin, M)
    SP = np.ascontiguousarray(inputs["SP"], dtype=f).reshape(B, Cs, HW)

    bn1s = (np.asarray(inputs["bn1_gamma"]) / np.sqrt(np.asarray(inputs["bn1_var"]) + EPS)).astype(f)
    bn1b = (np.asarray(inputs["bn1_beta"]) - np.asarray(inputs["bn1_mean"]) * bn1s).astype(f)
    bn2s = (np.asarray(inputs["bn2_gamma"]) / np.sqrt(np.asarray(inputs["bn2_var"]) + EPS)).astype(f)
    bn2b = (np.asarray(inputs["bn2_beta"]) - np.asarray(inputs["bn2_mean"]) * bn2s).astype(f)

    wpack = np.concatenate(
        [
            np.asarray(inputs["linKC_w"]).T,   # (128, 64)
            np.asarray(inputs["gnn_w"]).T,     # (128, 128)
            np.asarray(inputs["sp_w"]).T,      # (128, 128)
            np.asarray(inputs["back_w"]).T,    # (128, 128)
        ],
        axis=1,
    ).astype(bf)
    biases = np.stack([bn1s, bn1b,
                       np.asarray(inputs["gnn_b"], dtype=f),
                       np.asarray(inputs["sp_b"], dtype=f),
                       bn2s, bn2b], axis=1).astype(f)

    shared = {
        "st": np.ascontiguousarray(np.asarray(inputs["sp_adj"]).T).astype(bf),
        "w1t": np.ascontiguousarray(np.asarray(inputs["trans_w"]).T).astype(bf),
        "wnct": np.ascontiguousarray(np.asarray(inputs["linNC_w"]).T).astype(bf),
        "bnc": np.asarray(inputs["linNC_b"], dtype=f).reshape(1, Cs).astype(bf),
        "wpack": np.ascontiguousarray(wpack),
        "biases": np.ascontiguousarray(biases),
        "bkc": np.asarray(inputs["linKC_b"], dtype=f).reshape(Cs, 1),
    }
    in_maps = []
    for b in range(B):
        m = dict(shared)
        m["x"] = np.ascontiguousarray(x[b]).astype(bf)
        m["sp"] = np.ascontiguousarray(SP[b]).astype(bf)
        in_maps.append(m)
    return in_maps


def _get_nc():
    if "nc" not in _CACHE:
        _CACHE["nc"] = _build()
    return _CACHE["nc"]


def run_spmd(inputs, trace=False, trace_cores=None):
    """Build (cached), run on cores 0-7, return BassKernelResults."""
    from concourse.bass_utils import run_bass_kernel_spmd

    nc = _get_nc()
    in_maps = _host_prep(inputs)
    kwargs = {}
    if trace:
        kwargs = dict(trace=True, trace_cores=trace_cores or [0])
    return run_bass_kernel_spmd(nc, in_maps, core_ids=list(range(8)), **kwargs)


def kernel(**inputs):
    res = run_spmd(inputs)
    out = np.stack([r["out"].reshape(Co, N, N) for r in res.results])
    return out.astype(np.float32)


# revision 14
# speedup vs baseline: 1.3331x; 1.0251x over previous
"""Trainium2 Bass kernel for nn_AE_30142080483951 (gnn_message_passing).

Data-parallel over batch B=8 across 8 NeuronCores (one batch element per
core, weights replicated, no collectives).  Key restructuring vs the
reference:

  - The (M,M) affinity matrix A = SPf^T @ sigma @ SPf is rank-64, so
    A @ yT is computed as SPf^T @ (sigma @ (SPf @ yT)) without ever
    materializing A; the gnn linear is folded into the same low-rank chain.
  - softmax(sp_adj) @ yT is computed from the host-transposed adjacency
    ST = sp_adj.T streamed in (2304, 512)-column blocks: one DMA + one
    in-place ScalarE exp per block, the row-normalizer via a ones-matmul
    column sum, the division deferred to the (Ci, M) output.
  - BatchNorms are folded to per-channel scale/bias applied by ScalarE
    activations straight out of PSUM.
  - bf16 compute on the TensorEngine (rel tolerance 2e-2), fp32 PSUM
    accumulation and fp32 residual/activation chain.
"""

import numpy as np
from contextlib import ExitStack

EPS = 1e-5
B, N, Cs, Cin, Ci, Co = 8, 48, 64, 256, 128, 128
M = N * N            # 2304
MT = M // 128        # 18 token tiles
HW = (2 * N) * (2 * N)  # 9216
HWH = HW // 2        # 4608 (one image row-half per partition group)
CH = [(0, 512), (512, 512), (1024, 512), (1536, 512), (2048, 256)]

_CACHE = {}


def _build():
    import concourse.bacc as bacc_mod
    import concourse.mybir as mybir
    import concourse.tile as tile
    from concourse.bass import MemorySpace

    f32 = mybir.dt.float32
    bf = mybir.dt.bfloat16
    AF = mybir.ActivationFunctionType

    nc = bacc_mod.Bacc("TRN2")

    # ---- DRAM parameters (per-core shard; bf16 for matmul operands) ----
    x_d = nc.dram_tensor("x", [Cin, M], bf, kind="ExternalInput")
    sp_d = nc.dram_tensor("sp", [Cs, HW], bf, kind="ExternalInput")
    st_d = nc.dram_tensor("st", [M, M], bf, kind="ExternalInput")
    w1t_d = nc.dram_tensor("w1t", [Cin, Ci], bf, kind="ExternalInput")
    wnct_d = nc.dram_tensor("wnct", [M, Cs], bf, kind="ExternalInput")
    bnc_d = nc.dram_tensor("bnc", [1, Cs], bf, kind="ExternalInput")
    # packed (Ci, 448) = [wkct(64) | gnnwt(128) | spwt(128) | backwt(128)]
    wpack_d = nc.dram_tensor("wpack", [Ci, 448], bf, kind="ExternalInput")
    # packed (Ci, 6) = [bn1s bn1b gnnb spb bn2s bn2b]
    bias_d = nc.dram_tensor("biases", [Ci, 6], f32, kind="ExternalInput")
    bkc_d = nc.dram_tensor("bkc", [Cs, 1], f32, kind="ExternalInput")
    ident_d = nc.dram_tensor("ident", [128, 128], bf, kind="ExternalInput")
    sel_d = nc.dram_tensor("sel", [128, 2], bf, kind="ExternalInput")
    sel2_d = nc.dram_tensor("sel2", [2, 128], bf, kind="ExternalInput")
    out_d = nc.dram_tensor("out", [Co, M], f32, kind="ExternalOutput")

    tc = tile.TileContext(nc)
    with tc:
        with ExitStack() as ctx:
            ctx.enter_context(
                nc.allow_low_precision(reason="bf16 compute path, rel tol 2e-2")
            )
            singles = ctx.enter_context(tc.tile_pool(name="singles", bufs=1))
            psA = ctx.enter_context(
                tc.tile_pool(name="psA", bufs=6, space=MemorySpace.PSUM)
            )
            psS = ctx.enter_context(
                tc.tile_pool(name="psS", bufs=2, space=MemorySpace.PSUM)
            )

            # ---- persistent constants ----
            wpack_sb = singles.tile([Ci, 448], bf)
            nc.sync.dma_start(out=wpack_sb[:], in_=wpack_d[:, :])
            wkct_sb = wpack_sb[:, 0:64]
            gnnwt_sb = wpack_sb[:, 64:192]
            spwt_sb = wpack_sb[:, 192:320]
            backwt_sb = wpack_sb[:, 320:448]
            bias_sb = singles.tile([Ci, 6], f32)
            nc.sync.dma_start(out=bias_sb[:], in_=bias_d[:, :])
            bn1s_sb = bias_sb[:, 0:1]
            bn1b_sb = bias_sb[:, 1:2]
            gnnb_sb = bias_sb[:, 2:3]
            spb_sb = bias_sb[:, 3:4]
            bn2s_sb = bias_sb[:, 4:5]
            bn2b_sb = bias_sb[:, 5:6]
            bkc_sb = singles.tile([Cs, 1], f32)
            nc.sync.dma_start(out=bkc_sb[:], in_=bkc_d[:, :])
            bnc_sb = singles.tile([1, Cs], bf)
            nc.sync.dma_start(out=bnc_sb[:], in_=bnc_d[:, :])

            onesP = singles.tile([128, 128], bf)
            nc.vector.memset(onesP[:], 1.0)
            ones1 = onesP[0:1, :]

            # persistent activations
            spf_sb = singles.tile([Cs, M], bf)
            t_sb = singles.tile([Ci, M], bf)
            yT_sb = singles.tile([128, MT, Ci], bf)
            hg_sb = singles.tile([Cs, Ci], bf)

            # ---- phase 1: inputs -> spf, t, yT, low-rank chain -> hg ----
            with tc.tile_pool(name="phase1", bufs=1) as p1:
                ident = p1.tile([128, 128], bf)
                nc.sync.dma_start(out=ident[:], in_=ident_d[:, :])
                sel = p1.tile([128, 2], bf)
                nc.sync.dma_start(out=sel[:], in_=sel_d[:, :])
                sel2 = p1.tile([2, 128], bf)
                nc.sync.dma_start(out=sel2[:], in_=sel2_d[:, :])

                w1t_sb = p1.tile([128, 2, Ci], bf)
                nc.sync.dma_start(
                    out=w1t_sb[:],
                    in_=w1t_d[:, :].rearrange("(kt p) c -> p kt c", p=128),
                )
                x_sb = p1.tile([128, 2, M], bf)
                nc.sync.dma_start(
                    out=x_sb[:], in_=x_d[:, :].rearrange("(kt p) m -> p kt m", p=128)
                )
                wnct_sb = p1.tile([128, MT, Cs], bf)
                nc.sync.dma_start(
                    out=wnct_sb[:],
                    in_=wnct_d[:, :].rearrange("(ti p) c -> p ti c", p=128),
                )

                # t = relu(bn1(W1 @ x)), channel-major (128, 2304)
                for mo, mw in CH:
                    ps = psA.tile([128, 512], f32, tag="ps_big")
                    nc.tensor.matmul(
                        ps[:, :mw],
                        w1t_sb[:, 0, :],
                        x_sb[:, 0, mo : mo + mw],
                        start=True,
                        stop=False,
                    )
                    nc.tensor.matmul(
                        ps[:, :mw],
                        w1t_sb[:, 1, :],
                        x_sb[:, 1, mo : mo + mw],
                        start=False,
                        stop=True,
                    )
                    nc.scalar.activation(
                        t_sb[:, mo : mo + mw],
                        ps[:, :mw],
                        AF.Relu,
                        bias=bn1b_sb,
                        scale=bn1s_sb,
                    )

                # yT (token-major) via PE transposes
                for ti in range(MT):
                    ps = psS.tile([128, 128], bf, tag="ps_small")
                    nc.tensor.transpose(
                        ps[:], t_sb[:, ti * 128 : (ti + 1) * 128], ident[:]
                    )
                    nc.vector.tensor_copy(yT_sb[:, ti, :], ps[:])

                # SP branch: channel softmax + 2x2 maxpool
                # sp_sb partition p = h*64 + c (h = image row-half)
                sp_sb = p1.tile([128, HWH], bf)
                nc.sync.dma_start(out=sp_sb[0:64, :], in_=sp_d[:, 0:HWH])
                nc.sync.dma_start(out=sp_sb[64:128, :], in_=sp_d[:, HWH:HW])
                nc.scalar.activation(sp_sb[:], sp_sb[:], AF.Exp)
                # per-pixel channel sums via sel-matmul; fast reciprocal
                d_sbf = p1.tile([2, HWH], f32)
                d_bf = p1.tile([2, HWH], bf)
                for jo in range(0, HWH, 512):
                    ps = psS.tile([2, 512], f32, tag="ps_small")
                    nc.tensor.matmul(ps[:], sel[:], sp_sb[:, jo : jo + 512])
                    nc.vector.reciprocal_approx_fast(d_sbf[:, jo : jo + 512], ps[:])
                nc.vector.tensor_copy(d_bf[:], d_sbf[:])
                # broadcast 1/sums to all (h, c) partitions via selector matmul
                d_rep = p1.tile([128, HWH], bf)
                for jo in range(0, HWH, 512):
                    ps = psA.tile([128, 512], f32, tag="ps_big")
                    nc.tensor.matmul(ps[:], sel2[:], d_bf[:, jo : jo + 512])
                    nc.vector.tensor_copy(d_rep[:, jo : jo + 512], ps[:])
                nc.vector.tensor_mul(sp_sb[:], sp_sb[:], d_rep[:])
                # maxpool 2x2: f = rp*192 + dy*96 + qp*2 + dx
                pooled = p1.tile([128, 1152], bf)
                nc.vector.tensor_reduce(
                    out=pooled[:].rearrange("p (rp qp) -> p rp qp", rp=24),
                    in_=sp_sb[:].rearrange(
                        "p (rp dy qp dx) -> p rp qp dy dx", rp=24, dy=2, qp=48, dx=2
                    ),
                    axis=mybir.AxisListType.XY,
                    op=mybir.AluOpType.max,
                )
                # reassemble channel-major SPf (64, 2304)
                nc.sync.dma_start(out=spf_sb[:, 0:1152], in_=pooled[0:64, :])
                nc.sync.dma_start(out=spf_sb[:, 1152:2304], in_=pooled[64:128, :])

                # SPfT (token-major) via PE transposes
                spfT_sb = p1.tile([128, MT, Cs], bf)
                for ti in range(MT):
                    ps = psS.tile([128, 64], bf, tag="ps_small")
                    nc.tensor.transpose(
                        ps[:],
                        spf_sb[:, ti * 128 : (ti + 1) * 128],
                        ident[0:64, 0:64],
                    )
                    nc.vector.tensor_copy(spfT_sb[:, ti, :], ps[:])

                # yc (k, c) = t @ WncT + bnc
                ps_yc = psS.tile([128, Cs], f32, tag="ps_small")
                for ti in range(MT):
                    nc.tensor.matmul(
                        ps_yc[:],
                        yT_sb[:, ti, :],
                        wnct_sb[:, ti, :],
                        start=(ti == 0),
                        stop=False,
                    )
                nc.tensor.matmul(ps_yc[:], ones1, bnc_sb[:], start=False, stop=True)
                yc_sb = p1.tile([Ci, Cs], bf)
                nc.vector.tensor_copy(yc_sb[:], ps_yc[:])

                # sigT (d, c) = Wkc @ yc + bkc
                ps_sg = psS.tile([Cs, Cs], f32, tag="ps_small")
                nc.tensor.matmul(ps_sg[:], wkct_sb, yc_sb[:])
                sigT_sb = p1.tile([Cs, Cs], bf)
                nc.scalar.activation(sigT_sb[:], ps_sg[:], AF.Identity, bias=bkc_sb[:])

                # G (d, k) = SPf @ yT
                ps_g = psS.tile([Cs, Ci], f32, tag="ps_small")
                for ti in range(MT):
                    nc.tensor.matmul(
                        ps_g[:],
                        spfT_sb[:, ti, :],
                        yT_sb[:, ti, :],
                        start=(ti == 0),
                        stop=(ti == MT - 1),
                    )
                g_sb = p1.tile([Cs, Ci], bf)
                nc.vector.tensor_copy(g_sb[:], ps_g[:])

                # HT (k, c) = G.T @ sigT
                ps_ht = psS.tile([Ci, Cs], f32, tag="ps_small")
                nc.tensor.matmul(ps_ht[:], g_sb[:], sigT_sb[:])
                ht_sb = p1.tile([Ci, Cs], bf)
                nc.vector.tensor_copy(ht_sb[:], ps_ht[:])

                # HG (c, k2) = HT.T @ gnn_wT
                ps_hg = psS.tile([Cs, Ci], f32, tag="ps_small")
                nc.tensor.matmul(ps_hg[:], ht_sb[:], gnnwt_sb)
                nc.vector.tensor_copy(hg_sb[:], ps_hg[:])

            # ---- main chunk loop: se linear, sp branch, combine, back ----
            with (
                tc.tile_pool(name="stream", bufs=2) as stream,
                tc.tile_pool(name="chunks", bufs=2) as chunks,
            ):
                for mo, mw in CH:
                    # one DMA + one in-place exp for the whole column block
                    est = stream.tile([128, MT, 512], bf, tag="stj")
                    nc.sync.dma_start(
                        out=est[:, :, :mw],
                        in_=st_d[:, mo : mo + mw].rearrange(
                            "(i p) m -> p i m", p=128
                        ),
                    )
                    nc.scalar.activation(
                        est[:, :, :mw], est[:, :, :mw], AF.Exp
                    )

                    # relu_se chunk = relu(HG.T @ SPf + gnn_b)
                    ps_se = psA.tile([128, 512], f32, tag="ps_big")
                    nc.tensor.matmul(
                        ps_se[:, :mw], hg_sb[:], spf_sb[:, mo : mo + mw]
                    )
                    rse = chunks.tile([128, 512], f32, tag="rse")
                    nc.scalar.activation(
                        rse[:, :mw], ps_se[:, :mw], AF.Relu, bias=gnnb_sb
                    )

                    # sp_preT chunk: colsum matmuls, then main matmuls
                    ps_cs = psA.tile([128, 512], f32, tag="ps_big")
                    for i in range(MT):
                        nc.tensor.matmul(
                            ps_cs[:, :mw],
                            onesP[:],
                            est[:, i, :mw],
                            start=(i == 0),
                            stop=(i == MT - 1),
                        )
                    ps_sp = psA.tile([128, 512], f32, tag="ps_big")
                    for i in range(MT):
                        nc.tensor.matmul(
                            ps_sp[:, :mw],
                            yT_sb[:, i, :],
                            est[:, i, :mw],
                            start=(i == 0),
                            stop=(i == MT - 1),
                        )
                    rrep = chunks.tile([128, 512], f32, tag="rrep")
                    nc.vector.reciprocal_approx_fast(rrep[:, :mw], ps_cs[:, :mw])
                    spre = chunks.tile([128, 512], bf, tag="spre")
                    nc.vector.tensor_mul(spre[:, :mw], ps_sp[:, :mw], rrep[:, :mw])

                    # relu_sp chunk = relu(sp_w @ sp_preT + sp_b)
                    ps_sl = psA.tile([128, 512], f32, tag="ps_big")
                    nc.tensor.matmul(ps_sl[:, :mw], spwt_sb, spre[:, :mw])
                    y3a = chunks.tile([128, 512], f32, tag="y3a")
                    nc.scalar.activation(
                        y3a[:, :mw], ps_sl[:, :mw], AF.Relu, bias=spb_sb
                    )

                    # y3 = relu_se + relu_sp + 3t
                    nc.vector.tensor_add(y3a[:, :mw], y3a[:, :mw], rse[:, :mw])
                    y3b = chunks.tile([128, 512], bf, tag="y3b")
                    nc.vector.scalar_tensor_tensor(
                        out=y3b[:, :mw],
                        in0=t_sb[:, mo : mo + mw],
                        scalar=3.0,
                        in1=y3a[:, :mw],
                        op0=mybir.AluOpType.mult,
                        op1=mybir.AluOpType.add,
                    )

                    # back: relu(bn2(back_w @ y3))
                    ps_bk = psA.tile([128, 512], f32, tag="ps_big")
                    nc.tensor.matmul(ps_bk[:, :mw], backwt_sb, y3b[:, :mw])
                    ob = chunks.tile([128, 512], f32, tag="ob")
                    nc.scalar.activation(
                        ob[:, :mw],
                        ps_bk[:, :mw],
                        AF.Relu,
                        bias=bn2b_sb,
                        scale=bn2s_sb,
                    )
                    nc.sync.dma_start(out=out_d[:, mo : mo + mw], in_=ob[:, :mw])

    nc.finalize()
    return nc


def _host_prep(inputs):
    """Fold BNs, transpose weights, cast matmul operands to bf16, build
    the 8 per-core input maps (core b gets batch element b)."""
    import ml_dtypes

    f = np.float32
    bf = ml_dtypes.bfloat16
    x = np.ascontiguousarray(inputs["x"], dtype=f).reshape(B, Cin, M)
    SP = np.ascontiguousarray(inputs["SP"], dtype=f).reshape(B, Cs, HW)

    bn1s = (np.asarray(inputs["bn1_gamma"]) / np.sqrt(np.asarray(inputs["bn1_var"]) + EPS)).astype(f)
    bn1b = (np.asarray(inputs["bn1_beta"]) - np.asarray(inputs["bn1_mean"]) * bn1s).astype(f)
    bn2s = (np.asarray(inputs["bn2_gamma"]) / np.sqrt(np.asarray(inputs["bn2_var"]) + EPS)).astype(f)
    bn2b = (np.asarray(inputs["bn2_beta"]) - np.asarray(inputs["bn2_mean"]) * bn2s).astype(f)

    wpack = np.concatenate(
        [
            np.asarray(inputs["linKC_w"]).T,   # (128, 64)
            np.asarray(inputs["gnn_w"]).T,     # (128, 128)
            np.asarray(inputs["sp_w"]).T,      # (128, 128)
            np.asarray(inputs["back_w"]).T,    # (128, 128)
        ],
        axis=1,
    ).astype(bf)
    biases = np.stack([bn1s, bn1b,
                       np.asarray(inputs["gnn_b"], dtype=f),
                       np.asarray(inputs["sp_b"], dtype=f),
                       bn2s, bn2b], axis=1).astype(f)

    shared = {
        "st": np.ascontiguousarray(np.asarray(inputs["sp_adj"]).T).astype(bf),
        "w1t": np.ascontiguousarray(np.asarray(inputs["trans_w"]).T).astype(bf),
        "wnct": np.ascontiguousarray(np.asarray(inputs["linNC_w"]).T).astype(bf),
        "bnc": np.asarray(inputs["linNC_b"], dtype=f).reshape(1, Cs).astype(bf),
        "wpack": np.ascontiguousarray(wpack),
        "biases": np.ascontiguousarray(biases),
        "bkc": np.asarray(inputs["linKC_b"], dtype=f).reshape(Cs, 1),
        "ident": np.eye(128, dtype=f).astype(bf),
        "sel": np.repeat(np.eye(2, dtype=f), 64, axis=0).astype(bf),
        "sel2": np.repeat(np.eye(2, dtype=f), 64, axis=1).astype(bf),
    }
    in_maps = []
    for b in range(B):
        m = dict(shared)
        m["x"] = np.ascontiguousarray(x[b]).astype(bf)
        m["sp"] = np.ascontiguousarray(SP[b]).astype(bf)
        in_maps.append(m)
    return in_maps


def _get_nc():
    if "nc" not in _CACHE:
        _CACHE["nc"] = _build()
    return _CACHE["nc"]


def run_spmd(inputs, trace=False, trace_cores=None):
    """Build (cached), run on cores 0-7, return BassKernelResults."""
    from concourse.bass_utils import run_bass_kernel_spmd

    nc = _get_nc()
    in_maps = _host_prep(inputs)
    kwargs = {}
    if trace:
        kwargs = dict(trace=True, trace_cores=trace_cores or [0])
    return run_bass_kernel_spmd(nc, in_maps, core_ids=list(range(8)), **kwargs)


def kernel(**inputs):
    res = run_spmd(inputs)
    out = np.stack([r["out"].reshape(Co, N, N) for r in res.results])
    return out.astype(np.float32)


# revision 16
# speedup vs baseline: 1.4456x; 1.0844x over previous
"""Trainium2 Bass kernel for nn_AE_30142080483951 (gnn_message_passing).

Data-parallel over batch B=8 across 8 NeuronCores (one batch element per
core, weights replicated, no collectives).  Key restructuring vs the
reference:

  - The (M,M) affinity matrix A = SPf^T @ sigma @ SPf is rank-64, so
    A @ yT is computed as SPf^T @ (sigma @ (SPf @ yT)) without ever
    materializing A; the gnn linear is folded into the same low-rank chain.
  - softmax(sp_adj) @ yT is computed from the host-transposed adjacency
    ST = sp_adj.T streamed in (2304, 512)-column blocks: one DMA (on the
    otherwise-idle SWDGE queues) + one in-place ScalarE exp per block, the
    row-normalizer via a ones-matmul column sum, the division deferred to
    the (Ci, M) output.
  - BatchNorms are folded to per-channel scale/bias applied by ScalarE
    activations straight out of PSUM.
  - bf16 compute on the TensorEngine (rel tolerance 2e-2), fp32 PSUM
    accumulation and fp32 residual/activation chain.
"""

import numpy as np
from contextlib import ExitStack

EPS = 1e-5
B, N, Cs, Cin, Ci, Co = 8, 48, 64, 256, 128, 128
M = N * N            # 2304
MT = M // 128        # 18 token tiles
HW = (2 * N) * (2 * N)  # 9216
HWH = HW // 2        # 4608 (one image row-half per partition group)
CH = [(0, 512), (512, 512), (1024, 512), (1536, 512), (2048, 256)]

_CACHE = {}


def _build():
    import concourse.bacc as bacc_mod
    import concourse.mybir as mybir
    import concourse.tile as tile
    from concourse.bass import MemorySpace

    f32 = mybir.dt.float32
    bf = mybir.dt.bfloat16
    AF = mybir.ActivationFunctionType

    nc = bacc_mod.Bacc("TRN2")

    # ---- DRAM parameters (per-core shard; bf16 for matmul operands) ----
    x_d = nc.dram_tensor("x", [Cin, M], bf, kind="ExternalInput")
    sp_d = nc.dram_tensor("sp", [Cs, HW], bf, kind="ExternalInput")
    st_d = nc.dram_tensor("st", [M, M], bf, kind="ExternalInput")
    w1t_d = nc.dram_tensor("w1t", [Cin, Ci], bf, kind="ExternalInput")
    wnct_d = nc.dram_tensor("wnct", [M, Cs], bf, kind="ExternalInput")
    bnc_d = nc.dram_tensor("bnc", [1, Cs], bf, kind="ExternalInput")
    # packed (Ci, 448) = [wkct(64) | gnnwt(128) | spwt(128) | backwt(128)]
    wpack_d = nc.dram_tensor("wpack", [Ci, 448], bf, kind="ExternalInput")
    # packed (Ci, 6) = [bn1s bn1b gnnb spb bn2s bn2b]
    bias_d = nc.dram_tensor("biases", [Ci, 6], f32, kind="ExternalInput")
    bkc_d = nc.dram_tensor("bkc", [Cs, 1], f32, kind="ExternalInput")
    ident_d = nc.dram_tensor("ident", [128, 128], bf, kind="ExternalInput")
    sel_d = nc.dram_tensor("sel", [128, 2], bf, kind="ExternalInput")
    out_d = nc.dram_tensor("out", [Co, M], f32, kind="ExternalOutput")

    tc = tile.TileContext(nc)
    with tc:
        with ExitStack() as ctx:
            ctx.enter_context(
                nc.allow_low_precision(reason="bf16 compute path, rel tol 2e-2")
            )
            singles = ctx.enter_context(tc.tile_pool(name="singles", bufs=1))
            stream = ctx.enter_context(tc.tile_pool(name="stream", bufs=3))
            chunks = ctx.enter_context(tc.tile_pool(name="chunks", bufs=2))
            spres = ctx.enter_context(tc.tile_pool(name="spres", bufs=5))
            psA = ctx.enter_context(
                tc.tile_pool(name="psA", bufs=6, space=MemorySpace.PSUM)
            )
            psS = ctx.enter_context(
                tc.tile_pool(name="psS", bufs=2, space=MemorySpace.PSUM)
            )

            # ---- persistent constants ----
            wpack_sb = singles.tile([Ci, 448], bf)
            nc.sync.dma_start(out=wpack_sb[:], in_=wpack_d[:, :])
            wkct_sb = wpack_sb[:, 0:64]
            gnnwt_sb = wpack_sb[:, 64:192]
            spwt_sb = wpack_sb[:, 192:320]
            backwt_sb = wpack_sb[:, 320:448]
            bias_sb = singles.tile([Ci, 6], f32)
            nc.sync.dma_start(out=bias_sb[:], in_=bias_d[:, :])
            bn1s_sb = bias_sb[:, 0:1]
            bn1b_sb = bias_sb[:, 1:2]
            gnnb_sb = bias_sb[:, 2:3]
            spb_sb = bias_sb[:, 3:4]
            bn2s_sb = bias_sb[:, 4:5]
            bn2b_sb = bias_sb[:, 5:6]
            bkc_sb = singles.tile([Cs, 1], f32)
            nc.sync.dma_start(out=bkc_sb[:], in_=bkc_d[:, :])
            bnc_sb = singles.tile([1, Cs], bf)
            nc.sync.dma_start(out=bnc_sb[:], in_=bnc_d[:, :])
            onesP = singles.tile([128, 128], bf)
            nc.vector.memset(onesP[:], 1.0)
            ones1 = onesP[0:1, :]

            # persistent activations
            spf_sb = singles.tile([Cs, M], bf)
            t_sb = singles.tile([Ci, M], bf)
            yT_sb = singles.tile([128, MT, Ci], bf)
            hg_sb = singles.tile([Cs, Ci], bf)

            with tc.tile_pool(name="phase1", bufs=1) as p1:
                ident = p1.tile([128, 128], bf)
                nc.sync.dma_start(out=ident[:], in_=ident_d[:, :])
                sel = p1.tile([128, 2], bf)
                nc.sync.dma_start(out=sel[:], in_=sel_d[:, :])

                # SP input + exp first (independent of everything else)
                sp_sb = p1.tile([128, HWH], bf)
                nc.sync.dma_start(out=sp_sb[0:64, :], in_=sp_d[:, 0:HWH])
                nc.sync.dma_start(out=sp_sb[64:128, :], in_=sp_d[:, HWH:HW])
                nc.scalar.activation(sp_sb[:], sp_sb[:], AF.Exp)

                # ---- t = relu(bn1(W1 @ x)) ----
                w1t_sb = p1.tile([128, 2, Ci], bf)
                nc.sync.dma_start(
                    out=w1t_sb[:],
                    in_=w1t_d[:, :].rearrange("(kt p) c -> p kt c", p=128),
                )
                x_sb = p1.tile([128, 2, M], bf)
                nc.sync.dma_start(
                    out=x_sb[:], in_=x_d[:, :].rearrange("(kt p) m -> p kt m", p=128)
                )
                wnct_sb = p1.tile([128, MT, Cs], bf)
                nc.sync.dma_start(
                    out=wnct_sb[:],
                    in_=wnct_d[:, :].rearrange("(ti p) c -> p ti c", p=128),
                )
                for mo, mw in CH:
                    ps = psA.tile([128, 512], f32, tag="ps_big")
                    nc.tensor.matmul(
                        ps[:, :mw],
                        w1t_sb[:, 0, :],
                        x_sb[:, 0, mo : mo + mw],
                        start=True,
                        stop=False,
                    )
                    nc.tensor.matmul(
                        ps[:, :mw],
                        w1t_sb[:, 1, :],
                        x_sb[:, 1, mo : mo + mw],
                        start=False,
                        stop=True,
                    )
                    nc.scalar.activation(
                        t_sb[:, mo : mo + mw],
                        ps[:, :mw],
                        AF.Relu,
                        bias=bn1b_sb,
                        scale=bn1s_sb,
                    )

                # yT (token-major) via PE transposes
                for ti in range(MT):
                    ps = psS.tile([128, 128], bf, tag="ps_small")
                    nc.tensor.transpose(
                        ps[:], t_sb[:, ti * 128 : (ti + 1) * 128], ident[:]
                    )
                    nc.vector.tensor_copy(yT_sb[:, ti, :], ps[:])

                # ---- ST column-block prefetch + exp (SWDGE, idle queues) ----
                est_tiles = {}

                def prefetch(j):
                    mo, mw = CH[j]
                    est = stream.tile([128, MT, 512], bf, tag="stj")
                    nc.gpsimd.dma_start(
                        out=est[:, :, :mw],
                        in_=st_d[:, mo : mo + mw].rearrange(
                            "(i p) m -> p i m", p=128
                        ),
                    )
                    nc.scalar.activation(est[:, :, :mw], est[:, :, :mw], AF.Exp)
                    est_tiles[j] = est

                prefetch(0)
                prefetch(1)

                # ---- SP softmax normalizer + scale + maxpool -> spf ----
                d_sbf = p1.tile([2, HWH], f32)
                for jo in range(0, HWH, 512):
                    ps = psS.tile([2, 512], f32, tag="ps_small")
                    nc.tensor.matmul(ps[:], sel[:], sp_sb[:, jo : jo + 512])
                    nc.vector.reciprocal_approx_fast(d_sbf[:, jo : jo + 512], ps[:])
                d_dram = nc.dram_tensor("d_scratch", [2, HWH], f32, kind="Internal")
                nc.sync.dma_start(out=d_dram[:, :], in_=d_sbf[:])
                d_rep = p1.tile([128, HWH], bf)
                nc.gpsimd.dma_start(
                    out=d_rep[0:64, :], in_=d_dram[0:1, :].to_broadcast((64, HWH))
                )
                nc.gpsimd.dma_start(
                    out=d_rep[64:128, :], in_=d_dram[1:2, :].to_broadcast((64, HWH))
                )
                nc.vector.tensor_mul(sp_sb[:], sp_sb[:], d_rep[:])
                # maxpool 2x2: f = rp*192 + dy*96 + qp*2 + dx
                pooled = p1.tile([128, 1152], bf)
                nc.vector.tensor_reduce(
                    out=pooled[:].rearrange("p (rp qp) -> p rp qp", rp=24),
                    in_=sp_sb[:].rearrange(
                        "p (rp dy qp dx) -> p rp qp dy dx", rp=24, dy=2, qp=48, dx=2
                    ),
                    axis=mybir.AxisListType.XY,
                    op=mybir.AluOpType.max,
                )
                nc.sync.dma_start(out=spf_sb[:, 0:1152], in_=pooled[0:64, :])
                nc.sync.dma_start(out=spf_sb[:, 1152:2304], in_=pooled[64:128, :])

                # ---- sp-branch matmul pipeline over all column blocks ----
                spre_tiles = {}
                for j, (mo, mw) in enumerate(CH):
                    if j + 2 < len(CH):
                        prefetch(j + 2)
                    est = est_tiles[j]
                    ps_cs = psA.tile([128, 512], f32, tag="ps_big")
                    for i in range(MT):
                        nc.tensor.matmul(
                            ps_cs[:, :mw],
                            onesP[:],
                            est[:, i, :mw],
                            start=(i == 0),
                            stop=(i == MT - 1),
                        )
                    ps_sp = psA.tile([128, 512], f32, tag="ps_big")
                    for i in range(MT):
                        nc.tensor.matmul(
                            ps_sp[:, :mw],
                            yT_sb[:, i, :],
                            est[:, i, :mw],
                            start=(i == 0),
                            stop=(i == MT - 1),
                        )
                    rrep = chunks.tile([128, 512], f32, tag="rrep")
                    nc.vector.reciprocal_approx_fast(rrep[:, :mw], ps_cs[:, :mw])
                    spre = spres.tile([128, 512], bf, tag="spre")
                    nc.vector.tensor_mul(spre[:, :mw], ps_sp[:, :mw], rrep[:, :mw])
                    spre_tiles[j] = spre

                # ---- low-rank affinity chain (overlaps the loop above) ----
                spfT_sb = p1.tile([128, MT, Cs], bf)
                for ti in range(MT):
                    ps = psS.tile([128, 64], bf, tag="ps_small")
                    nc.tensor.transpose(
                        ps[:],
                        spf_sb[:, ti * 128 : (ti + 1) * 128],
                        ident[0:64, 0:64],
                    )
                    nc.vector.tensor_copy(spfT_sb[:, ti, :], ps[:])

                ps_yc = psS.tile([128, Cs], f32, tag="ps_small")
                for ti in range(MT):
                    nc.tensor.matmul(
                        ps_yc[:],
                        yT_sb[:, ti, :],
                        wnct_sb[:, ti, :],
                        start=(ti == 0),
                        stop=False,
                    )
                nc.tensor.matmul(ps_yc[:], ones1, bnc_sb[:], start=False, stop=True)
                yc_sb = p1.tile([Ci, Cs], bf)
                nc.vector.tensor_copy(yc_sb[:], ps_yc[:])

                ps_sg = psS.tile([Cs, Cs], f32, tag="ps_small")
                nc.tensor.matmul(ps_sg[:], wkct_sb, yc_sb[:])
                sigT_sb = p1.tile([Cs, Cs], bf)
                nc.scalar.activation(sigT_sb[:], ps_sg[:], AF.Identity, bias=bkc_sb[:])

                ps_g = psS.tile([Cs, Ci], f32, tag="ps_small")
                for ti in range(MT):
                    nc.tensor.matmul(
                        ps_g[:],
                        spfT_sb[:, ti, :],
                        yT_sb[:, ti, :],
                        start=(ti == 0),
                        stop=(ti == MT - 1),
                    )
                g_sb = p1.tile([Cs, Ci], bf)
                nc.vector.tensor_copy(g_sb[:], ps_g[:])

                ps_ht = psS.tile([Ci, Cs], f32, tag="ps_small")
                nc.tensor.matmul(ps_ht[:], g_sb[:], sigT_sb[:])
                ht_sb = p1.tile([Ci, Cs], bf)
                nc.vector.tensor_copy(ht_sb[:], ps_ht[:])

                ps_hg = psS.tile([Cs, Ci], f32, tag="ps_small")
                nc.tensor.matmul(ps_hg[:], ht_sb[:], gnnwt_sb)
                nc.vector.tensor_copy(hg_sb[:], ps_hg[:])

                # ---- chunk tails: se linear, combine, back, store ----
                for j, (mo, mw) in enumerate(CH):
                    ps_se = psA.tile([128, 512], f32, tag="ps_big")
                    nc.tensor.matmul(
                        ps_se[:, :mw], hg_sb[:], spf_sb[:, mo : mo + mw]
                    )
                    rse = chunks.tile([128, 512], f32, tag="rse")
                    nc.scalar.activation(
                        rse[:, :mw], ps_se[:, :mw], AF.Relu, bias=gnnb_sb
                    )

                    ps_sl = psA.tile([128, 512], f32, tag="ps_big")
                    nc.tensor.matmul(
                        ps_sl[:, :mw], spwt_sb, spre_tiles[j][:, :mw]
                    )
                    y3a = chunks.tile([128, 512], f32, tag="y3a")
                    nc.scalar.activation(
                        y3a[:, :mw], ps_sl[:, :mw], AF.Relu, bias=spb_sb
                    )

                    nc.vector.tensor_add(y3a[:, :mw], y3a[:, :mw], rse[:, :mw])
                    y3b = chunks.tile([128, 512], bf, tag="y3b")
                    nc.vector.scalar_tensor_tensor(
                        out=y3b[:, :mw],
                        in0=t_sb[:, mo : mo + mw],
                        scalar=3.0,
                        in1=y3a[:, :mw],
                        op0=mybir.AluOpType.mult,
                        op1=mybir.AluOpType.add,
                    )

                    ps_bk = psA.tile([128, 512], f32, tag="ps_big")
                    nc.tensor.matmul(ps_bk[:, :mw], backwt_sb, y3b[:, :mw])
                    ob = chunks.tile([128, 512], f32, tag="ob")
                    nc.scalar.activation(
                        ob[:, :mw],
                        ps_bk[:, :mw],
                        AF.Relu,
                        bias=bn2b_sb,
                        scale=bn2s_sb,
                    )
                    nc.sync.dma_start(out=out_d[:, mo : mo + mw], in_=ob[:, :mw])

    nc.finalize()
    return nc


def _host_prep(inputs):
    """Fold BNs, transpose weights, cast matmul operands to bf16, build
    the 8 per-core input maps (core b gets batch element b)."""
    import ml_dtypes

    f = np.float32
    bf = ml_dtypes.bfloat16
    x = np.ascontiguousarray(inputs["x"], dtype=f).reshape(B, Cin, M)
    SP = np.ascontiguousarray(inputs["SP"], dtype=f).reshape(B, Cs, HW)

    bn1s = (np.asarray(inputs["bn1_gamma"]) / np.sqrt(np.asarray(inputs["bn1_var"]) + EPS)).astype(f)
    bn1b = (np.asarray(inputs["bn1_beta"]) - np.asarray(inputs["bn1_mean"]) * bn1s).astype(f)
    bn2s = (np.asarray(inputs["bn2_gamma"]) / np.sqrt(np.asarray(inputs["bn2_var"]) + EPS)).astype(f)
    bn2b = (np.asarray(inputs["bn2_beta"]) - np.asarray(inputs["bn2_mean"]) * bn2s).astype(f)

    wpack = np.concatenate(
        [
            np.asarray(inputs["linKC_w"]).T,   # (128, 64)
            np.asarray(inputs["gnn_w"]).T,     # (128, 128)
            np.asarray(inputs["sp_w"]).T,      # (128, 128)
            np.asarray(inputs["back_w"]).T,    # (128, 128)
        ],
        axis=1,
    ).astype(bf)
    biases = np.stack([bn1s, bn1b,
                       np.asarray(inputs["gnn_b"], dtype=f),
                       np.asarray(inputs["sp_b"], dtype=f),
                       bn2s, bn2b], axis=1).astype(f)

    shared = {
        "st": np.ascontiguousarray(np.asarray(inputs["sp_adj"]).T).astype(bf),
        "w1t": np.ascontiguousarray(np.asarray(inputs["trans_w"]).T).astype(bf),
        "wnct": np.ascontiguousarray(np.asarray(inputs["linNC_w"]).T).astype(bf),
        "bnc": np.asarray(inputs["linNC_b"], dtype=f).reshape(1, Cs).astype(bf),
        "wpack": np.ascontiguousarray(wpack),
        "biases": np.ascontiguousarray(biases),
        "bkc": np.asarray(inputs["linKC_b"], dtype=f).reshape(Cs, 1),
        "ident": np.eye(128, dtype=f).astype(bf),
        "sel": np.repeat(np.eye(2, dtype=f), 64, axis=0).astype(bf),
    }
    in_maps = []
    for b in range(B):
        m = dict(shared)
        m["x"] = np.ascontiguousarray(x[b]).astype(bf)
        m["sp"] = np.ascontiguousarray(SP[b]).astype(bf)
        in_maps.append(m)
    return in_maps


def _get_nc():
    if "nc" not in _CACHE:
        _CACHE["nc"] = _build()
    return _CACHE["nc"]


def run_spmd(inputs, trace=False, trace_cores=None):
    """Build (cached), run on cores 0-7, return BassKernelResults."""
    from concourse.bass_utils import run_bass_kernel_spmd

    nc = _get_nc()
    in_maps = _host_prep(inputs)
    kwargs = {}
    if trace:
        kwargs = dict(trace=True, trace_cores=trace_cores or [0])
    return run_bass_kernel_spmd(nc, in_maps, core_ids=list(range(8)), **kwargs)


def kernel(**inputs):
    res = run_spmd(inputs)
    out = np.stack([r["out"].reshape(Co, N, N) for r in res.results])
    return out.astype(np.float32)


# revision 18
# speedup vs baseline: 1.9620x; 1.3573x over previous
"""Trainium2 Bass kernel for nn_AE_30142080483951 (gnn_message_passing).

Data-parallel over batch B=8 across 8 NeuronCores (one batch element per
core, weights replicated, no collectives).  Key restructuring vs the
reference:

  - The (M,M) affinity matrix A = SPf^T @ sigma @ SPf is rank-64, so
    A @ yT is computed as SPf^T @ (sigma @ (SPf @ yT)) without ever
    materializing A; the gnn linear is folded into the same low-rank chain.
  - softmax(sp_adj) @ yT is computed from the host-transposed adjacency
    ST = sp_adj.T streamed in (2304, 512)-column blocks: one DMA (on the
    otherwise-idle SWDGE queues) + one in-place ScalarE exp per block, the
    row-normalizer via a ones-matmul column sum, the division deferred to
    the (Ci, M) output.
  - BatchNorms are folded to per-channel scale/bias applied by ScalarE
    activations straight out of PSUM.
  - bf16 compute on the TensorEngine (rel tolerance 2e-2), fp32 PSUM
    accumulation and fp32 residual/activation chain.
"""

import numpy as np
from contextlib import ExitStack

EPS = 1e-5
B, N, Cs, Cin, Ci, Co = 8, 48, 64, 256, 128, 128
M = N * N            # 2304
MT = M // 128        # 18 token tiles
HW = (2 * N) * (2 * N)  # 9216
HWH = HW // 2        # 4608 (one image row-half per partition group)
CH = [(0, 512), (512, 512), (1024, 512), (1536, 512), (2048, 256)]

_CACHE = {}


def _build():
    import concourse.bacc as bacc_mod
    import concourse.mybir as mybir
    import concourse.tile as tile
    from concourse.bass import MemorySpace

    f32 = mybir.dt.float32
    bf = mybir.dt.bfloat16
    AF = mybir.ActivationFunctionType

    nc = bacc_mod.Bacc("TRN2", num_swdge_queues=4)

    # ---- DRAM parameters (per-core shard; bf16 for matmul operands) ----
    x_d = nc.dram_tensor("x", [Cin, M], bf, kind="ExternalInput")
    sp_d = nc.dram_tensor("sp", [Cs, HW], bf, kind="ExternalInput")
    st_d = nc.dram_tensor("st", [M, M], bf, kind="ExternalInput")
    w1t_d = nc.dram_tensor("w1t", [Cin, Ci], bf, kind="ExternalInput")
    wnct_d = nc.dram_tensor("wnct", [M, Cs], bf, kind="ExternalInput")
    bnc_d = nc.dram_tensor("bnc", [1, Cs], bf, kind="ExternalInput")
    # packed (Ci, 448) = [wkct(64) | gnnwt(128) | spwt(128) | backwt(128)]
    wpack_d = nc.dram_tensor("wpack", [Ci, 448], bf, kind="ExternalInput")
    # packed (Ci, 6) = [bn1s bn1b gnnb spb bn2s bn2b]
    bias_d = nc.dram_tensor("biases", [Ci, 6], f32, kind="ExternalInput")
    bkc_d = nc.dram_tensor("bkc", [Cs, 1], f32, kind="ExternalInput")
    ident_d = nc.dram_tensor("ident", [128, 128], bf, kind="ExternalInput")
    sel_d = nc.dram_tensor("sel", [128, 2], bf, kind="ExternalInput")
    out_d = nc.dram_tensor("out", [Co, M], f32, kind="ExternalOutput")

    tc = tile.TileContext(nc)
    with tc:
        with ExitStack() as ctx:
            ctx.enter_context(
                nc.allow_low_precision(reason="bf16 compute path, rel tol 2e-2")
            )
            singles = ctx.enter_context(tc.tile_pool(name="singles", bufs=1))
            stream = ctx.enter_context(tc.tile_pool(name="stream", bufs=3))
            chunks = ctx.enter_context(tc.tile_pool(name="chunks", bufs=2))
            psA = ctx.enter_context(
                tc.tile_pool(name="psA", bufs=4, space=MemorySpace.PSUM)
            )
            psS = ctx.enter_context(
                tc.tile_pool(name="psS", bufs=2, space=MemorySpace.PSUM)
            )

            # ---- persistent constants ----
            wpack_sb = singles.tile([Ci, 448], bf)
            nc.sync.dma_start(out=wpack_sb[:], in_=wpack_d[:, :])
            wkct_sb = wpack_sb[:, 0:64]
            gnnwt_sb = wpack_sb[:, 64:192]
            spwt_sb = wpack_sb[:, 192:320]
            backwt_sb = wpack_sb[:, 320:448]
            bias_sb = singles.tile([Ci, 6], f32)
            nc.sync.dma_start(out=bias_sb[:], in_=bias_d[:, :])
            bn1s_sb = bias_sb[:, 0:1]
            bn1b_sb = bias_sb[:, 1:2]
            gnnb_sb = bias_sb[:, 2:3]
            spb_sb = bias_sb[:, 3:4]
            bn2s_sb = bias_sb[:, 4:5]
            bn2b_sb = bias_sb[:, 5:6]
            bkc_sb = singles.tile([Cs, 1], f32)
            nc.sync.dma_start(out=bkc_sb[:], in_=bkc_d[:, :])
            bnc_sb = singles.tile([1, Cs], bf)
            nc.sync.dma_start(out=bnc_sb[:], in_=bnc_d[:, :])
            onesP = singles.tile([128, 128], bf)
            nc.vector.memset(onesP[:], 1.0)
            ones1 = onesP[0:1, :]

            # persistent activations
            spf_sb = singles.tile([Cs, M], bf)
            t_sb = singles.tile([Ci, M], bf)
            yT_sb = singles.tile([128, MT, Ci], bf)
            hg_sb = singles.tile([Cs, Ci], bf)

            with tc.tile_pool(name="phase1", bufs=1) as p1:
                ident = p1.tile([128, 128], bf)
                nc.sync.dma_start(out=ident[:], in_=ident_d[:, :])
                sel = p1.tile([128, 2], bf)
                nc.sync.dma_start(out=sel[:], in_=sel_d[:, :])

                # SP input + exp first (independent of everything else)
                sp_sb = p1.tile([128, HWH], bf)
                nc.sync.dma_start(out=sp_sb[0:64, :], in_=sp_d[:, 0:HWH])
                nc.sync.dma_start(out=sp_sb[64:128, :], in_=sp_d[:, HWH:HW])
                nc.scalar.activation(sp_sb[:], sp_sb[:], AF.Exp)

                # ---- t = relu(bn1(W1 @ x)) ----
                w1t_sb = p1.tile([128, 2, Ci], bf)
                nc.sync.dma_start(
                    out=w1t_sb[:],
                    in_=w1t_d[:, :].rearrange("(kt p) c -> p kt c", p=128),
                )
                x_sb = p1.tile([128, 2, M], bf)
                nc.sync.dma_start(
                    out=x_sb[:], in_=x_d[:, :].rearrange("(kt p) m -> p kt m", p=128)
                )
                wnct_sb = p1.tile([128, MT, Cs], bf)
                nc.sync.dma_start(
                    out=wnct_sb[:],
                    in_=wnct_d[:, :].rearrange("(ti p) c -> p ti c", p=128),
                )
                for mo, mw in CH:
                    ps = psA.tile([128, 512], f32, tag="ps_big")
                    nc.tensor.matmul(
                        ps[:, :mw],
                        w1t_sb[:, 0, :],
                        x_sb[:, 0, mo : mo + mw],
                        start=True,
                        stop=False,
                    )
                    nc.tensor.matmul(
                        ps[:, :mw],
                        w1t_sb[:, 1, :],
                        x_sb[:, 1, mo : mo + mw],
                        start=False,
                        stop=True,
                    )
                    nc.scalar.activation(
                        t_sb[:, mo : mo + mw],
                        ps[:, :mw],
                        AF.Relu,
                        bias=bn1b_sb,
                        scale=bn1s_sb,
                    )

                # yT (token-major) via PE transposes
                for ti in range(MT):
                    ps = psS.tile([128, 128], bf, tag="ps_small")
                    nc.tensor.transpose(
                        ps[:], t_sb[:, ti * 128 : (ti + 1) * 128], ident[:]
                    )
                    nc.vector.tensor_copy(yT_sb[:, ti, :], ps[:])

                # ---- ST column-block prefetch + exp (SWDGE, idle queues) ----
                est_tiles = {}

                HT2 = MT // 2

                def prefetch(j):
                    mo, mw = CH[j]
                    est = stream.tile([128, MT, 512], bf, tag="stj")
                    for h in range(2):
                        nc.sync.dma_start(
                            out=est[:, h * HT2 : (h + 1) * HT2, :mw],
                            in_=st_d[
                                h * (M // 2) : (h + 1) * (M // 2), mo : mo + mw
                            ].rearrange("(i p) m -> p i m", p=128),
                        )
                        nc.scalar.activation(
                            est[:, h * HT2 : (h + 1) * HT2, :mw],
                            est[:, h * HT2 : (h + 1) * HT2, :mw],
                            AF.Exp,
                        )
                    est_tiles[j] = est

                prefetch(0)
                prefetch(1)

                # ---- SP softmax normalizer + scale + maxpool -> spf ----
                d_sbf = p1.tile([2, HWH], f32)
                for jo in range(0, HWH, 512):
                    ps = psS.tile([2, 512], f32, tag="ps_small")
                    nc.tensor.matmul(ps[:], sel[:], sp_sb[:, jo : jo + 512])
                    nc.vector.reciprocal_approx_fast(d_sbf[:, jo : jo + 512], ps[:])
                d_dram = nc.dram_tensor("d_scratch", [2, HWH], f32, kind="Internal")
                nc.gpsimd.dma_start(out=d_dram[:, :], in_=d_sbf[:])
                d_rep = p1.tile([128, HWH], bf)
                nc.gpsimd.dma_start(
                    out=d_rep[0:64, :], in_=d_dram[0:1, :].to_broadcast((64, HWH))
                )
                nc.gpsimd.dma_start(
                    out=d_rep[64:128, :], in_=d_dram[1:2, :].to_broadcast((64, HWH))
                )
                pooled = p1.tile([128, 1152], bf)
                for hf in range(2):
                    sl = slice(hf * (HWH // 2), (hf + 1) * (HWH // 2))
                    nc.vector.tensor_mul(sp_sb[:, sl], sp_sb[:, sl], d_rep[:, sl])
                    # maxpool 2x2: f = rp*192 + dy*96 + qp*2 + dx
                    nc.vector.tensor_reduce(
                        out=pooled[:, hf * 576 : (hf + 1) * 576].rearrange(
                            "p (rp qp) -> p rp qp", rp=12
                        ),
                        in_=sp_sb[:, sl].rearrange(
                            "p (rp dy qp dx) -> p rp qp dy dx",
                            rp=12, dy=2, qp=48, dx=2,
                        ),
                        axis=mybir.AxisListType.XY,
                        op=mybir.AluOpType.max,
                    )
                nc.sync.dma_start(out=spf_sb[:, 0:1152], in_=pooled[0:64, :])
                nc.sync.dma_start(out=spf_sb[:, 1152:2304], in_=pooled[64:128, :])

                # ---- sp-branch matmul pipeline over all column blocks ----
                spre_sb = singles.tile([Ci, M], bf)
                for j, (mo, mw) in enumerate(CH):
                    if j + 2 < len(CH):
                        prefetch(j + 2)
                    est = est_tiles[j]
                    # column sums (M=1 weight: negligible LDWEIGHTS)
                    ps_cs = psS.tile([1, 512], f32, tag="ps_cs")
                    for i in range(MT):
                        nc.tensor.matmul(
                            ps_cs[:, :mw],
                            onesP[:, 0:1],
                            est[:, i, :mw],
                            start=(i == 0),
                            stop=(i == MT - 1),
                        )
                    rr1 = chunks.tile([1, 512], f32, tag="rr1")
                    nc.vector.reciprocal_approx_fast(rr1[:, :mw], ps_cs[:, :mw])
                    rr1b = chunks.tile([1, 512], bf, tag="rr1b")
                    nc.vector.tensor_copy(rr1b[:, :mw], rr1[:, :mw])
                    # broadcast 1/colsum to 128 partitions via K=1 matmul
                    ps_rr = psS.tile([128, 512], f32, tag="ps_cs")
                    nc.tensor.matmul(ps_rr[:, :mw], ones1, rr1b[:, :mw])
                    rrep = chunks.tile([128, 512], bf, tag="rrep")
                    nc.vector.tensor_copy(rrep[:, :mw], ps_rr[:, :mw])
                    ps_sp = psA.tile([128, 512], f32, tag="ps_big")
                    for i in range(MT):
                        nc.tensor.matmul(
                            ps_sp[:, :mw],
                            yT_sb[:, i, :],
                            est[:, i, :mw],
                            start=(i == 0),
                            stop=(i == MT - 1),
                        )
                    nc.vector.tensor_mul(
                        spre_sb[:, mo : mo + mw], ps_sp[:, :mw], rrep[:, :mw]
                    )

                # ---- low-rank affinity chain (overlaps the loop above) ----
                spfT_sb = p1.tile([128, MT, Cs], bf)
                for ti in range(MT):
                    ps = psS.tile([128, 64], bf, tag="ps_small")
                    nc.tensor.transpose(
                        ps[:],
                        spf_sb[:, ti * 128 : (ti + 1) * 128],
                        ident[0:64, 0:64],
                    )
                    nc.vector.tensor_copy(spfT_sb[:, ti, :], ps[:])

                ps_yc = psS.tile([128, Cs], f32, tag="ps_small")
                for ti in range(MT):
                    nc.tensor.matmul(
                        ps_yc[:],
                        yT_sb[:, ti, :],
                        wnct_sb[:, ti, :],
                        start=(ti == 0),
                        stop=False,
                    )
                nc.tensor.matmul(ps_yc[:], ones1, bnc_sb[:], start=False, stop=True)
                yc_sb = p1.tile([Ci, Cs], bf)
                nc.vector.tensor_copy(yc_sb[:], ps_yc[:])

                ps_sg = psS.tile([Cs, Cs], f32, tag="ps_small")
                nc.tensor.matmul(ps_sg[:], wkct_sb, yc_sb[:])
                sigT_sb = p1.tile([Cs, Cs], bf)
                nc.scalar.activation(sigT_sb[:], ps_sg[:], AF.Identity, bias=bkc_sb[:])

                ps_g = psS.tile([Cs, Ci], f32, tag="ps_small")
                for ti in range(MT):
                    nc.tensor.matmul(
                        ps_g[:],
                        spfT_sb[:, ti, :],
                        yT_sb[:, ti, :],
                        start=(ti == 0),
                        stop=(ti == MT - 1),
                    )
                g_sb = p1.tile([Cs, Ci], bf)
                nc.vector.tensor_copy(g_sb[:], ps_g[:])

                ps_ht = psS.tile([Ci, Cs], f32, tag="ps_small")
                nc.tensor.matmul(ps_ht[:], g_sb[:], sigT_sb[:])
                ht_sb = p1.tile([Ci, Cs], bf)
                nc.vector.tensor_copy(ht_sb[:], ps_ht[:])

                ps_hg = psS.tile([Cs, Ci], f32, tag="ps_small")
                nc.tensor.matmul(ps_hg[:], ht_sb[:], gnnwt_sb)
                nc.vector.tensor_copy(hg_sb[:], ps_hg[:])

                # ---- tails: se linear, combine, back, store ----
                rse_sb = p1.tile([Ci, M], f32)
                y3a_sb = p1.tile([Ci, M], f32)
                y3b_sb = p1.tile([Ci, M], bf)
                ob_sb = p1.tile([Co, M], f32)
                for mo, mw in CH:
                    ps_se = psA.tile([128, 512], f32, tag="ps_big")
                    nc.tensor.matmul(
                        ps_se[:, :mw], hg_sb[:], spf_sb[:, mo : mo + mw]
                    )
                    nc.scalar.activation(
                        rse_sb[:, mo : mo + mw], ps_se[:, :mw], AF.Relu,
                        bias=gnnb_sb,
                    )
                for mo, mw in CH:
                    ps_sl = psA.tile([128, 512], f32, tag="ps_big")
                    nc.tensor.matmul(
                        ps_sl[:, :mw], spwt_sb, spre_sb[:, mo : mo + mw]
                    )
                    nc.scalar.activation(
                        y3a_sb[:, mo : mo + mw], ps_sl[:, :mw], AF.Relu,
                        bias=spb_sb,
                    )
                nc.vector.tensor_add(y3a_sb[:], y3a_sb[:], rse_sb[:])
                nc.vector.scalar_tensor_tensor(
                    out=y3b_sb[:],
                    in0=t_sb[:],
                    scalar=3.0,
                    in1=y3a_sb[:],
                    op0=mybir.AluOpType.mult,
                    op1=mybir.AluOpType.add,
                )
                for mo, mw in CH:
                    ps_bk = psA.tile([128, 512], f32, tag="ps_big")
                    nc.tensor.matmul(
                        ps_bk[:, :mw], backwt_sb, y3b_sb[:, mo : mo + mw]
                    )
                    nc.scalar.activation(
                        ob_sb[:, mo : mo + mw],
                        ps_bk[:, :mw],
                        AF.Relu,
                        bias=bn2b_sb,
                        scale=bn2s_sb,
                    )
                    nc.sync.dma_start(
                        out=out_d[:, mo : mo + mw], in_=ob_sb[:, mo : mo + mw]
                    )

    nc.finalize()
    return nc


def _host_prep(inputs):
    """Fold BNs, transpose weights, cast matmul operands to bf16, build
    the 8 per-core input maps (core b gets batch element b)."""
    import ml_dtypes

    f = np.float32
    bf = ml_dtypes.bfloat16
    x = np.ascontiguousarray(inputs["x"], dtype=f).reshape(B, Cin, M)
    SP = np.ascontiguousarray(inputs["SP"], dtype=f).reshape(B, Cs, HW)

    bn1s = (np.asarray(inputs["bn1_gamma"]) / np.sqrt(np.asarray(inputs["bn1_var"]) + EPS)).astype(f)
    bn1b = (np.asarray(inputs["bn1_beta"]) - np.asarray(inputs["bn1_mean"]) * bn1s).astype(f)
    bn2s = (np.asarray(inputs["bn2_gamma"]) / np.sqrt(np.asarray(inputs["bn2_var"]) + EPS)).astype(f)
    bn2b = (np.asarray(inputs["bn2_beta"]) - np.asarray(inputs["bn2_mean"]) * bn2s).astype(f)

    wpack = np.concatenate(
        [
            np.asarray(inputs["linKC_w"]).T,   # (128, 64)
            np.asarray(inputs["gnn_w"]).T,     # (128, 128)
            np.asarray(inputs["sp_w"]).T,      # (128, 128)
            np.asarray(inputs["back_w"]).T,    # (128, 128)
        ],
        axis=1,
    ).astype(bf)
    biases = np.stack([bn1s, bn1b,
                       np.asarray(inputs["gnn_b"], dtype=f),
                       np.asarray(inputs["sp_b"], dtype=f),
                       bn2s, bn2b], axis=1).astype(f)

    shared = {
        "st": np.ascontiguousarray(np.asarray(inputs["sp_adj"]).T).astype(bf),
        "w1t": np.ascontiguousarray(np.asarray(inputs["trans_w"]).T).astype(bf),
        "wnct": np.ascontiguousarray(np.asarray(inputs["linNC_w"]).T).astype(bf),
        "bnc": np.asarray(inputs["linNC_b"], dtype=f).reshape(1, Cs).astype(bf),
        "wpack": np.ascontiguousarray(wpack),
        "biases": np.ascontiguousarray(biases),
        "bkc": np.asarray(inputs["linKC_b"], dtype=f).reshape(Cs, 1),
        "ident": np.eye(128, dtype=f).astype(bf),
        "sel": np.repeat(np.eye(2, dtype=f), 64, axis=0).astype(bf),
    }
    in_maps = []
    for b in range(B):
        m = dict(shared)
        m["x"] = np.ascontiguousarray(x[b]).astype(bf)
        m["sp"] = np.ascontiguousarray(SP[b]).astype(bf)
        in_maps.append(m)
    return in_maps


def _get_nc():
    if "nc" not in _CACHE:
        _CACHE["nc"] = _build()
    return _CACHE["nc"]


def run_spmd(inputs, trace=False, trace_cores=None):
    """Build (cached), run on cores 0-7, return BassKernelResults."""
    from concourse.bass_utils import run_bass_kernel_spmd

    nc = _get_nc()
    in_maps = _host_prep(inputs)
    kwargs = {}
    if trace:
        kwargs = dict(trace=True, trace_cores=trace_cores or [0])
    return run_bass_kernel_spmd(nc, in_maps, core_ids=list(range(8)), **kwargs)


def kernel(**inputs):
    res = run_spmd(inputs)
    out = np.stack([r["out"].reshape(Co, N, N) for r in res.results])
    return out.astype(np.float32)
